# revision 1
# baseline (speedup 1.0000x reference)
"""PointerNet additive-attention scores kernel for Trainium2 (8 NeuronCores).

Math (reference):
    kt[k,n,h] = key[k,n,:] @ w1_w[h,:] + w1_b[h]
    vt[v,n,h] = value[v,n,:] @ w2_w[h,:] + w2_b[h]
    xi[k,v,n] = sum_h v_w[h] * tanh(kt[k,n,h] + vt[v,n,h]) + v_b
    S[k,n]    = sum_v exp(xi[k,v,n]) * mask[v,n];  S==0 -> 1
    out[k,n,v] = xi[k,v,n] - log(S[k,n])

Sharding: data-parallel over batch N (16) across 8 cores, NLOC=2 batch items
per core. Each core computes its (Lk, Lv, 2) slab independently; host slices
inputs / concatenates outputs.

Per-core dataflow (fully unrolled, Tile framework):
  - Host pre-transposes key/value to [n, d, k] and w1/w2 to [d, h] so every
    device DMA is contiguous; input DMAs are spread across 5 engine queues
    so the prologue is fed within ~3 us.
  - PE (fp32): ktT[h,k] / vtT[h,v] per (n, h-chunk); bias added via a c=1
    ones-row matmul into the same PSUM accumulation group.
  - DVE tensor_scalar (bf16 in0/out + per-partition f32 scalar -> 4x mode):
    X[:, k-slice] = ktT_bf + vtT[:, v]  - the (k,v) broadcast add, one
    instruction per v, free dim 128 (the k axis).
  - ACT: wide tanh over [128, 4096] tiles - the roofline engine
    (Lk*Lv*Nloc*H/128 = 65536 lane-cycles @ 1.2 GHz ~ 55 us/core).
  - PE (bf16): xi columns - T_v [128h, 128k] is the *stationary* operand
    (FWL-accelerated LDWEIGHTS), rhs = v_w column [128, 1], giving
    out = psum_xi[:, v] (full 128 partitions, so the 32-strip base-partition
    rule is satisfied); accumulated over the two h-chunks; seeded with v_b
    via a c=1 ones matmul (one seed per PSUM bank: start=True clears
    has_written bank-wide).
  - Epilogue in [k, (n v)] layout: exp -> mask multiply (mask rows
    replicated across partitions with c=1 ones matmuls) -> free-dim reduce
    -> S==0 guard -> log via DVE polynomial (avoids the ~2.7us ACT
    table-set switch to natural_log) -> per-partition subtract -> one
    contiguous DMA out.
"""

import numpy as np

LK, LV, N, D, H = 128, 128, 16, 256, 256
NCORES = 8
NLOC = N // NCORES  # batch items per core
VB = 32  # v-block per X tile -> ACT free dim 4096
NVB = LV // VB

# ln(m) on m in [1, 2]: degree-6 least-squares fit (max err ~1.5e-6).
_LN_COEF = None


def _ln_coef():
    global _LN_COEF
    if _LN_COEF is None:
        xs = np.linspace(1.0, 2.0, 20001)
        _LN_COEF = np.polynomial.Polynomial.fit(xs, np.log(xs), 6).convert().coef
    return _LN_COEF


_CACHE = {}


def _build_program(reps=1):
    from contextlib import ExitStack

    import concourse.bacc as bacc
    import concourse.mybir as mybir
    import concourse.tile as tile

    f32 = mybir.dt.float32
    i32 = mybir.dt.int32
    bf16 = mybir.dt.bfloat16
    AF = mybir.ActivationFunctionType
    ALU = mybir.AluOpType

    nc = bacc.Bacc("TRN2", target_bir_lowering=False, debug=False)

    keyT = nc.dram_tensor("keyT", [NLOC, D, LK], bf16, kind="ExternalInput").ap()
    valT = nc.dram_tensor("valT", [NLOC, D, LV], bf16, kind="ExternalInput").ap()
    w1T = nc.dram_tensor("w1T", [D, H], bf16, kind="ExternalInput").ap()
    w2T = nc.dram_tensor("w2T", [D, H], bf16, kind="ExternalInput").ap()
    b12r = nc.dram_tensor("b12r", [1, H], f32, kind="ExternalInput").ap()
    vwr = nc.dram_tensor("vwr", [1, H], f32, kind="ExternalInput").ap()
    vbrow = nc.dram_tensor("vbrow", [1, NLOC * LV], f32, kind="ExternalInput").ap()
    maskr = nc.dram_tensor("maskr", [NLOC, LV], f32, kind="ExternalInput").ap()
    scores = nc.dram_tensor("scores", [LK, NLOC, LV], f32, kind="ExternalOutput").ap()

    cf = [float(c) for c in _ln_coef()]
    LN2 = float(np.log(2.0))

    with tile.TileContext(nc) as tc, ExitStack() as ctx:
        const = ctx.enter_context(tc.tile_pool(name="const", bufs=1 if reps == 1 else 2))
        ppre = ctx.enter_context(tc.tile_pool(name="ppre", bufs=2, space="PSUM"))
        pacc = ctx.enter_context(tc.tile_pool(name="pacc", bufs=1, space="PSUM"))
        pepi = ctx.enter_context(tc.tile_pool(name="pepi", bufs=1, space="PSUM"))
        xpool = ctx.enter_context(tc.tile_pool(name="xpool", bufs=6))
        tpool = ctx.enter_context(tc.tile_pool(name="tpool", bufs=6))
        epool = ctx.enter_context(tc.tile_pool(name="epool", bufs=2))

        for _rep in range(reps):
            # ---- input loads, spread over DMA queues so prologue feeds fast ----
            keyT_v = keyT.rearrange("n (c p) k -> p n c k", p=128)
            valT_v = valT.rearrange("n (c p) k -> p n c k", p=128)
            keyT_sb = const.tile([128, NLOC, 2, LK], bf16)  # (d%128, n, d//128, k)
            valT_sb = const.tile([128, NLOC, 2, LV], bf16)
            w1T_sb = const.tile([128, 2, H], bf16)  # (d%128, d//128, h)
            w2T_sb = const.tile([128, 2, H], bf16)
            # sync queue: b1 + w1 + key; scalar queue: b2 + w2 + value;
            # gpsimd (SWDGE): the rest of the small tensors
            nc.sync.dma_start(out=w1T_sb, in_=w1T.rearrange("(c p) h -> p c h", p=128))
            nc.scalar.dma_start(
                out=w2T_sb, in_=w2T.rearrange("(c p) h -> p c h", p=128)
            )
            b12_sb = const.tile([1, H], f32)
            nc.sync.dma_start(out=b12_sb, in_=b12r)
            nc.sync.dma_start(out=keyT_sb[:, 0], in_=keyT_v[:, 0])
            nc.scalar.dma_start(out=valT_sb[:, 0], in_=valT_v[:, 0])
            nc.sync.dma_start(out=keyT_sb[:, 1], in_=keyT_v[:, 1])
            nc.scalar.dma_start(out=valT_sb[:, 1], in_=valT_v[:, 1])
            # v_w as per-partition columns [128, hc]
            vwcol_f32 = const.tile([128, 2], f32)
            nc.gpsimd.dma_start(
                out=vwcol_f32, in_=vwr.rearrange("o (c p) -> p (o c)", p=128)
            )
            vb_sb = const.tile([1, NLOC * LV], f32)
            nc.gpsimd.dma_start(out=vb_sb, in_=vbrow)
            mask_sb = []
            for n in range(NLOC):
                m = const.tile([1, LV], f32, tag=f"mask{n}")
                nc.gpsimd.dma_start(out=m, in_=maskr[n : n + 1, :])
                mask_sb.append(m)
            vw_bf = const.tile([128, 2], bf16)

            ones = const.tile([1, 512], f32)
            nc.vector.memset(ones, 1.0)

            # ---- xi accumulator ([128k, n, 128v] packed into one PSUM bank) ----
            xi_t = pacc.tile([LK, NLOC, LV], f32, tag="xi")

            # b12 = (w1_b + w2_b) as per-partition columns [128, 2]: row DMA
            # + two c=1 matmuls (avoids a 128-descriptor strided DMA).
            b12_ps = pepi.tile([128, 2], f32, tag="b12ps")
            for hc in range(2):
                nc.tensor.matmul(
                    out=b12_ps[:, hc : hc + 1],
                    lhsT=b12_sb[:, hc * 128 : (hc + 1) * 128],
                    rhs=ones[:, :1],
                    start=True,
                    stop=True,
                )
            b12c_sb = const.tile([128, 2], f32)
            nc.vector.tensor_copy(b12c_sb, b12_ps)

            # ---- prologue matmuls: ktT/vtT into PSUM per (n, hc) ----
            # The PSUM->SBUF copies are deferred into the main loop so the
            # first TS block isn't queued behind all four (n, hc) copies on
            # the in-order DVE.
            ktT_bf = const.tile([128, NLOC, 2, LK], bf16)  # (h%128, n, hc, k)
            vtT_sb = const.tile([128, NLOC, 2, LV], f32)
            pk_t, pv_t = {}, {}
            for n in range(NLOC):
                for hc in range(2):
                    hsl = slice(hc * 128, (hc + 1) * 128)
                    pk = ppre.tile([128, LK], f32, tag="pk")
                    for dc in range(2):
                        nc.tensor.matmul(
                            out=pk,
                            lhsT=w1T_sb[:, dc, hsl],
                            rhs=keyT_sb[:, n, dc, :],
                            start=(dc == 0),
                            stop=(dc == 1),
                        )
                    pk_t[(n, hc)] = pk

                    pv = ppre.tile([128, LV], f32, tag="pv")
                    for dc in range(2):
                        nc.tensor.matmul(
                            out=pv,
                            lhsT=w2T_sb[:, dc, hsl],
                            rhs=valT_sb[:, n, dc, :],
                            start=(dc == 0),
                            stop=(dc == 1),
                        )
                    pv_t[(n, hc)] = pv

            # seed xi with v_b everywhere (one start=True per bank: start
            # clears has_written bank-wide); emitted after the prologue so PE
            # reaches the kt/vt matmuls first.
            nc.tensor.matmul(
                out=xi_t.rearrange("k n v -> k (n v)"),
                lhsT=ones[:, :LK],
                rhs=vb_sb,
                start=True,
                stop=True,
            )

            # ln() constants for the DVE log (shared by both epilogues)
            c23 = const.tile([128, 1], i32, tag="c23")
            nc.vector.memset(c23, 23)
            cmant = const.tile([128, 1], i32, tag="cmant")
            nc.vector.memset(cmant, 0x007FFFFF)
            cexp1 = const.tile([128, 1], i32, tag="cexp1")
            nc.vector.memset(cexp1, 0x3F800000)

            def epilogue(n):
                # [k, v] layout; S/logS are per-partition columns.
                nc.tensor.matmul(
                    out=pm_t[:, n, :],
                    lhsT=ones[:, :LK],
                    rhs=mask_sb[n],
                    start=True,
                    stop=True,
                )
                e_sb = epool.tile([LK, LV], f32, tag="e")
                nc.scalar.activation(e_sb, xi_t[:, n, :], AF.Exp)
                me = epool.tile([LK, LV], f32, tag="me")
                nc.vector.tensor_tensor(me, e_sb, pm_t[:, n, :], op=ALU.mult)
                S = epool.tile([LK, 1], f32, tag="S")
                nc.vector.reduce_sum(S, me, axis=mybir.AxisListType.X)
                Sg = epool.tile([LK, 1], f32, tag="Sg")
                # Sg = (S == 0 ? 1 : 0) + S  == reference's where(S==0, 1, S)
                nc.vector.scalar_tensor_tensor(
                    out=Sg, in0=S, scalar=0.0, in1=S, op0=ALU.is_equal, op1=ALU.add
                )
                # logS = ln(Sg): exponent/mantissa split + deg-6 poly, all DVE
                # (avoids the ACT natural_log table-set switch).
                xu = Sg.bitcast(i32)
                e_i = epool.tile([LK, 1], i32, tag="e_i")
                nc.vector.tensor_tensor(e_i, xu, c23, op=ALU.logical_shift_right)
                e_f = epool.tile([LK, 1], f32, tag="e_f")
                nc.vector.tensor_copy(e_f, e_i)  # int -> float convert
                m_i = epool.tile([LK, 1], i32, tag="m_i")
                nc.vector.tensor_tensor(m_i, xu, cmant, op=ALU.bitwise_and)
                nc.vector.tensor_tensor(m_i, m_i, cexp1, op=ALU.bitwise_or)
                m = m_i.bitcast(f32)  # mantissa in [1, 2)
                # Estrin: p = (c0+c1 m) + m2*((c2+c3 m) + m2*(c4+c5 m + c6 m2))
                m2 = epool.tile([LK, 1], f32, tag="m2")
                nc.vector.tensor_tensor(m2, m, m, op=ALU.mult)
                u = epool.tile([LK, 1], f32, tag="u")
                nc.vector.tensor_scalar(
                    out=u, in0=m, scalar1=cf[1], scalar2=cf[0], op0=ALU.mult, op1=ALU.add
                )
                vq = epool.tile([LK, 1], f32, tag="vq")
                nc.vector.tensor_scalar(
                    out=vq, in0=m, scalar1=cf[3], scalar2=cf[2], op0=ALU.mult, op1=ALU.add
                )
                w = epool.tile([LK, 1], f32, tag="w")
                nc.vector.tensor_scalar(
                    out=w, in0=m, scalar1=cf[5], scalar2=cf[4], op0=ALU.mult, op1=ALU.add
                )
                w2 = epool.tile([LK, 1], f32, tag="w2")
                nc.vector.scalar_tensor_tensor(
                    out=w2, in0=m2, scalar=cf[6], in1=w, op0=ALU.mult, op1=ALU.add
                )
                q2 = epool.tile([LK, 1], f32, tag="q2")
                nc.vector.scalar_tensor_tensor(
                    out=q2, in0=m2, scalar=1.0, in1=w2, op0=ALU.mult, op1=ALU.mult
                )
                nc.vector.tensor_tensor(q2, q2, vq, op=ALU.add)
                acc = epool.tile([LK, 1], f32, tag="acc")
                nc.vector.scalar_tensor_tensor(
                    out=acc, in0=m2, scalar=1.0, in1=q2, op0=ALU.mult, op1=ALU.mult
                )
                nc.vector.tensor_tensor(acc, acc, u, op=ALU.add)
                esc = epool.tile([LK, 1], f32, tag="esc")
                nc.vector.tensor_scalar(
                    out=esc, in0=e_f, scalar1=LN2, scalar2=-127.0 * LN2,
                    op0=ALU.mult, op1=ALU.add,
                )
                logS = epool.tile([LK, 1], f32, tag="logS")
                nc.vector.tensor_tensor(logS, esc, acc, op=ALU.add)
                sc = epool.tile([LK, LV], f32, tag="sc")
                nc.vector.tensor_scalar_sub(sc, xi_t[:, n, :], logS)
                nc.sync.dma_start(out=scores[:, n, :], in_=sc)

            pm_t = pepi.tile([LK, NLOC, LV], f32, tag="pm")

            # ---- main loop (block sizes ramp at the ends to shrink the
            # pipeline fill and the final PE/epilogue tail) ----
            RAMP_UP = [4, 4, 8, 16, 32, 32, 32]
            RAMP_DN = [32, 32, 32, 16, 8, 8]
            FLAT = [32, 32, 32, 32]
            for n in range(NLOC):
                for hc in range(2):
                    first = n == 0 and hc == 0
                    last = n == NLOC - 1 and hc == 1
                    blocks = RAMP_UP if first else (RAMP_DN if last else FLAT)
                    # deferred prologue copies (both biases fused into vt)
                    nc.vector.tensor_copy(ktT_bf[:, n, hc, :], pk_t[(n, hc)])
                    nc.vector.tensor_scalar_add(
                        vtT_sb[:, n, hc, :], pv_t[(n, hc)], b12c_sb[:, hc : hc + 1]
                    )
                    v0 = 0
                    for blk in blocks:
                        X = xpool.tile([128, blk, LK], bf16, tag="X")
                        for j in range(blk):
                            nc.vector.tensor_scalar_add(
                                X[:, j, :],
                                ktT_bf[:, n, hc, :],
                                vtT_sb[:, n, hc, v0 + j : v0 + j + 1],
                            )
                        if first and v0 == 0:
                            nc.vector.tensor_copy(vw_bf, vwcol_f32)
                        T = tpool.tile([128, blk, LK], bf16, tag="T")
                        nc.scalar.activation(T, X, AF.Tanh)
                        for j in range(blk):
                            nc.tensor.matmul(
                                out=xi_t[:, n, v0 + j : v0 + j + 1],
                                lhsT=T[:, j, :],
                                rhs=vw_bf[:, hc : hc + 1],
                                start=False,
                                stop=(hc == 1),
                                skip_group_check=True,
                            )
                        v0 += blk
                if n == 0:
                    epilogue(0)
            epilogue(NLOC - 1)

    nc.compile()
    return nc


def _get_program(reps=1):
    if reps not in _CACHE:
        _CACHE[reps] = _build_program(reps)
    return _CACHE[reps]


def _make_in_maps(key, value, mask, w1_w, w1_b, w2_w, w2_b, v_w, v_b):
    import ml_dtypes

    bf = ml_dtypes.bfloat16
    key = np.asarray(key, dtype=np.float32)
    value = np.asarray(value, dtype=np.float32)
    mask_f = np.asarray(mask).astype(np.float32)
    w1T_np = np.ascontiguousarray(np.asarray(w1_w, np.float32).T).astype(bf)  # [D, H]
    w2T_np = np.ascontiguousarray(np.asarray(w2_w, np.float32).T).astype(bf)
    b12r_np = (np.asarray(w1_b, np.float32) + np.asarray(w2_b, np.float32)).reshape(
        1, H
    )
    vwr_np = np.asarray(v_w, np.float32).reshape(1, H)
    vb_np = np.full(
        (1, NLOC * LV), np.float32(np.asarray(v_b).reshape(-1)[0]), np.float32
    )

    in_maps = []
    for c in range(NCORES):
        ns = slice(c * NLOC, (c + 1) * NLOC)
        keyT_c = np.ascontiguousarray(key[:, ns, :].transpose(1, 2, 0)).astype(bf)
        valT_c = np.ascontiguousarray(value[:, ns, :].transpose(1, 2, 0)).astype(bf)
        maskr_c = np.ascontiguousarray(mask_f[:, ns].T)  # [NLOC, LV]
        in_maps.append(
            {
                "keyT": keyT_c,
                "valT": valT_c,
                "w1T": w1T_np,
                "w2T": w2T_np,
                "b12r": b12r_np,
                "vwr": vwr_np,
                "vbrow": vb_np,
                "maskr": maskr_c,
            }
        )
    return in_maps


def kernel(**inputs):
    from concourse.bass_utils import run_bass_kernel_spmd

    nc = _get_program()
    in_maps = _make_in_maps(**inputs)
    res = run_bass_kernel_spmd(nc, in_maps, core_ids=list(range(NCORES)))
    out = np.empty((LK, N, LV), np.float32)
    for c in range(NCORES):
        out[:, c * NLOC : (c + 1) * NLOC, :] = res.results[c]["scores"]
    return out



# revision 11
# speedup vs baseline: 2.6658x; 2.6658x over previous
"""PointerNet additive-attention scores kernel for Trainium2 (8 NeuronCores).

Math (reference):
    kt[k,n,h] = key[k,n,:] @ w1_w[h,:]
    vt[v,n,h] = value[v,n,:] @ w2_w[h,:] + (w1_b[h] + w2_b[h])
    xi[k,v,n] = sum_h v_w[h] * tanh(kt + vt) + v_b
    S[k,n]    = sum_v exp(xi) * mask[v,n];  S==0 -> 1
    out[k,n,v] = xi - log(S)

Key trick: tanh is replaced by a rank-R trigonometric expansion
    tanh(x) ~= sum_r c_r sin(w_r x),   w_r = (2r+1) w0   (midpoint lattice)
so the (k,v) outer broadcast becomes R pairs of rank-H matmuls on PE:
    sin(w_r(kt+vt)) = sin(w_r kt) cos(w_r vt) + cos(w_r kt) sin(w_r vt)
ACT evaluates only the base pair sin/cos(w0 *) on kt/vt-sized tensors
(args stay inside ACT's valid sin range [-pi, pi]; cos = sin(pi/2 - w0 x));
DVE builds the odd harmonics 3w0, 5w0, ... with Chebyshev three-term
recurrences:
    S_{r+1} = 2 cos(2 w0 x) S_r - S_{r-1}   (same for the cos family)
using only tensor_tensor (2x mode) + tensor_scalar (4x) ops. v_w is folded
into the kt-side of the base pair ONCE (a fixed per-partition scale commutes
through the linear recurrence); the per-rank c_r is an immediate-scalar 4x
tensor_scalar. PE accumulates all 2R matmul terms per (n, h-chunk) into one
PSUM bank seeded with v_b; the vt bias rides a c=1 ones-row matmul in the
prologue so ACT reads kt/vt straight from PSUM.

Sharding: data-parallel over batch N (16) across 8 cores, NLOC=2 per core.

Epilogue: exp on ACT (its exp_and_others table load overlaps the DVE
ladder), mask replicated via c=1 ones matmuls, reduce + S==0 guard on DVE,
ln via DVE polynomial (avoids a second ACT table switch on the tail),
per-partition subtract, one DMA out.
"""

import numpy as np

LK, LV, N, D, H = 128, 128, 16, 256, 256
NCORES = 8
NLOC = N // NCORES
R = 5  # expansion rank (number of sin terms)
XFIT = 6.5  # fit domain for tanh ~= sum_r c_r sin((2r+1) w0 x)

# aux row layout: [b12 (H) | vw (H) | vb (NLOC*LV) | mask (NLOC*LV)]
AUX_B12, AUX_VW = 0, H
AUX_VB = 2 * H
AUX_MASK = 2 * H + NLOC * LV
AUX_LEN = 2 * H + 2 * NLOC * LV

_FIT = None


def _fit_ladder():
    """Least-squares fit of tanh on [0, XFIT] with the midpoint sine lattice.
    Returns (w0, coefs[R])."""
    global _FIT
    if _FIT is None:
        xs = np.linspace(0, XFIT, 3001)
        y = np.tanh(xs)
        best = None
        for dlt in np.linspace(2.5 / R, 7.5 / R, 160):
            om = (np.arange(R) + 0.5) * dlt
            A = np.sin(np.outer(xs, om))
            c, *_ = np.linalg.lstsq(A, y, rcond=None)
            e = np.abs(A @ c - y).max()
            if best is None or e < best[0]:
                best = (e, dlt, c)
        _FIT = (best[1] / 2.0, best[2])  # w0 = d/2
    return _FIT


# ln(m) on m in [1, 2]: degree-3 least-squares fit (max err ~2e-4).
_LN_COEF = None


def _ln_coef():
    global _LN_COEF
    if _LN_COEF is None:
        xs = np.linspace(1.0, 2.0, 20001)
        _LN_COEF = np.polynomial.Polynomial.fit(xs, np.log(xs), 3).convert().coef
    return _LN_COEF


_CACHE = {}


def _build_program(reps=1):
    from contextlib import ExitStack

    import concourse.bacc as bacc
    import concourse.mybir as mybir
    import concourse.tile as tile

    f32 = mybir.dt.float32
    i32 = mybir.dt.int32
    bf16 = mybir.dt.bfloat16
    AF = mybir.ActivationFunctionType
    ALU = mybir.AluOpType

    w0, coef = _fit_ladder()
    cf = [float(c) for c in _ln_coef()]
    LN2 = float(np.log(2.0))
    PI2 = float(np.pi / 2.0)

    nc = bacc.Bacc("TRN2", target_bir_lowering=False, debug=False)

    keyT = nc.dram_tensor("keyT", [NLOC, D, LK], bf16, kind="ExternalInput").ap()
    valT = nc.dram_tensor("valT", [NLOC, D, LV], bf16, kind="ExternalInput").ap()
    w1T = nc.dram_tensor("w1T", [D, H], bf16, kind="ExternalInput").ap()
    w2T = nc.dram_tensor("w2T", [D, H], bf16, kind="ExternalInput").ap()
    auxr = nc.dram_tensor("auxr", [1, AUX_LEN], f32, kind="ExternalInput").ap()
    scores = nc.dram_tensor("scores", [LK, NLOC, LV], f32, kind="ExternalOutput").ap()

    with tile.TileContext(nc) as tc, ExitStack() as ctx:
        const = ctx.enter_context(tc.tile_pool(name="const", bufs=1 if reps == 1 else 2))
        psum = ctx.enter_context(tc.tile_pool(name="psum", bufs=1, space="PSUM"))
        lpool = ctx.enter_context(tc.tile_pool(name="lpool", bufs=4))
        apool = ctx.enter_context(tc.tile_pool(name="apool", bufs=3))
        epool = ctx.enter_context(tc.tile_pool(name="epool", bufs=2))

        for _rep in range(reps):
            # ---- input DMAs: 4 big on sync/scalar queues + 1 aux on gpsimd ----
            keyT_sb = const.tile([128, NLOC, 2, LK], bf16)  # (d%128, n, d//128, k)
            valT_sb = const.tile([128, NLOC, 2, LV], bf16)
            w1T_sb = const.tile([128, 2, H], bf16)  # (d%128, d//128, h)
            w2T_sb = const.tile([128, 2, H], bf16)
            nc.sync.dma_start(out=w1T_sb, in_=w1T.rearrange("(c p) h -> p c h", p=128))
            nc.scalar.dma_start(out=w2T_sb, in_=w2T.rearrange("(c p) h -> p c h", p=128))
            nc.sync.dma_start(
                out=keyT_sb, in_=keyT.rearrange("n (c p) k -> p n c k", p=128)
            )
            nc.scalar.dma_start(
                out=valT_sb, in_=valT.rearrange("n (c p) k -> p n c k", p=128)
            )
            aux_sb = const.tile([1, AUX_LEN], f32)
            nc.gpsimd.dma_start(out=aux_sb, in_=auxr)

            ones = const.tile([1, 512], f32)
            nc.vector.memset(ones, 1.0)
            pi2col = const.tile([128, 1], f32, tag="pi2")
            nc.vector.memset(pi2col, PI2)

            # ---- PSUM layout (5 banks) ----
            ktps = psum.tile([128, NLOC, 2, LK], f32, tag="ktps")  # (h%128, n, hc, k)
            vtps = psum.tile([128, NLOC, 2, LV], f32, tag="vtps")
            xi_t = psum.tile([LK, NLOC, LV], f32, tag="xi")
            pm_t = psum.tile([LK, NLOC, LV], f32, tag="pm")
            vw_ps = psum.tile([128, 2], f32, tag="vwps")

            # ---- prologue matmuls: kt first (unblocks ACT), then vt+bias ----
            for n in range(NLOC):
                for hc in range(2):
                    hsl = slice(hc * 128, (hc + 1) * 128)
                    first = n == 0 and hc == 0
                    for dc in range(2):
                        nc.tensor.matmul(
                            out=ktps[:, n, hc, :],
                            lhsT=w1T_sb[:, dc, hsl],
                            rhs=keyT_sb[:, n, dc, :],
                            start=(dc == 0 and first),
                            stop=(dc == 1 and n == NLOC - 1 and hc == 1),
                            skip_group_check=True,
                        )
            for n in range(NLOC):
                for hc in range(2):
                    hsl = slice(hc * 128, (hc + 1) * 128)
                    first = n == 0 and hc == 0
                    for dc in range(2):
                        nc.tensor.matmul(
                            out=vtps[:, n, hc, :],
                            lhsT=w2T_sb[:, dc, hsl],
                            rhs=valT_sb[:, n, dc, :],
                            start=(dc == 0 and first),
                            stop=False,
                            skip_group_check=True,
                        )
                    # bias fold: vt += b12[h] (outer product with ones row)
                    nc.tensor.matmul(
                        out=vtps[:, n, hc, :],
                        lhsT=aux_sb[:, AUX_B12 + hc * 128 : AUX_B12 + (hc + 1) * 128],
                        rhs=ones[:, :LV],
                        start=False,
                        stop=(n == NLOC - 1 and hc == 1),
                        skip_group_check=True,
                    )

            # vw as per-partition columns [128, hc]
            for hc in range(2):
                nc.tensor.matmul(
                    out=vw_ps[:, hc : hc + 1],
                    lhsT=aux_sb[:, AUX_VW + hc * 128 : AUX_VW + (hc + 1) * 128],
                    rhs=ones[:, :1],
                    start=(hc == 0),
                    stop=(hc == 1),
                    skip_group_check=True,
                )
            vw_sb = const.tile([128, 2], f32)
            nc.vector.tensor_copy(vw_sb, vw_ps)

            # seed xi with v_b (start=True clears the bank)
            nc.tensor.matmul(
                out=xi_t.rearrange("k n v -> k (n v)"),
                lhsT=ones[:, :LK],
                rhs=aux_sb[:, AUX_VB : AUX_VB + NLOC * LV],
                start=True,
                stop=False,
                skip_group_check=True,
            )
            # mask rows replicated across partitions
            for n in range(NLOC):
                nc.tensor.matmul(
                    out=pm_t[:, n, :],
                    lhsT=ones[:, :LK],
                    rhs=aux_sb[:, AUX_MASK + n * LV : AUX_MASK + (n + 1) * LV],
                    start=(n == 0),
                    stop=(n == NLOC - 1),
                    skip_group_check=True,
                )

            # ---- ACT base pair: sin/cos(w0 x) straight from PSUM ----
            # ladder tiles [128, side(kt=0/vt=1), n, hc, 128] bf16
            S0 = lpool.tile([128, 2, NLOC, 2, 128], bf16, tag="S0")
            C0 = lpool.tile([128, 2, NLOC, 2, 128], bf16, tag="C0")
            nc.scalar.activation(S0[:, 0], ktps, AF.Sin, scale=w0)
            nc.scalar.activation(S0[:, 1], vtps, AF.Sin, scale=w0)
            nc.scalar.activation(C0[:, 0], ktps, AF.Sin, bias=pi2col, scale=-w0)
            nc.scalar.activation(C0[:, 1], vtps, AF.Sin, bias=pi2col, scale=-w0)

            # ---- DVE ladder scaffolding (from RAW S0, before the vw fold) ----
            # Cd = cos(2 w0 x) = 1 - 2 S0^2 ; Cd2/Cd1/Cdm = 2Cd / 2Cd+1 / 2Cd-1
            T0 = lpool.tile([128, 2, NLOC, 2, 128], bf16, tag="T0")
            nc.vector.tensor_tensor(T0, S0, S0, op=ALU.mult)
            # fold vw into the kt side of the base pair (in place); the
            # recurrence is linear, so the scale propagates to every rank.
            for hc in range(2):
                nc.vector.tensor_scalar_mul(
                    S0[:, 0, :, hc, :], S0[:, 0, :, hc, :], vw_sb[:, hc : hc + 1]
                )
                nc.vector.tensor_scalar_mul(
                    C0[:, 0, :, hc, :], C0[:, 0, :, hc, :], vw_sb[:, hc : hc + 1]
                )
            Cd2 = lpool.tile([128, 2, NLOC, 2, 128], bf16, tag="Cd2")
            nc.vector.tensor_scalar(
                out=Cd2, in0=T0, scalar1=-4.0, scalar2=2.0, op0=ALU.mult, op1=ALU.add
            )
            Cd1 = lpool.tile([128, 2, NLOC, 2, 128], bf16, tag="Cd1")
            nc.vector.tensor_scalar(
                out=Cd1, in0=T0, scalar1=-4.0, scalar2=3.0, op0=ALU.mult, op1=ALU.add
            )
            Cdm = lpool.tile([128, 2, NLOC, 2, 128], bf16, tag="Cdm")
            nc.vector.tensor_scalar(
                out=Cdm, in0=T0, scalar1=-4.0, scalar2=1.0, op0=ALU.mult, op1=ALU.add
            )

            def fold_and_matmul(r, Sr, Cr):
                """Scale the (vw-prefolded) kt side by the immediate c_r and
                emit the two matmul terms per (n, hc)."""
                cr = float(coef[r])
                As = apool.tile([128, NLOC, 2, 128], bf16, tag="As")
                Ac = apool.tile([128, NLOC, 2, 128], bf16, tag="Ac")
                eng = nc.vector
                eng.tensor_scalar(
                    out=As, in0=Sr[:, 0], scalar1=cr, scalar2=0.0,
                    op0=ALU.mult, op1=ALU.add,
                )
                eng.tensor_scalar(
                    out=Ac, in0=Cr[:, 0], scalar1=cr, scalar2=0.0,
                    op0=ALU.mult, op1=ALU.add,
                )
                last = r == R - 1
                for n in range(NLOC):
                    for hc in range(2):
                        nc.tensor.matmul(
                            out=xi_t[:, n, :],
                            lhsT=As[:, n, hc, :],
                            rhs=Cr[:, 1, n, hc, :],
                            start=False,
                            stop=False,
                            skip_group_check=True,
                        )
                        nc.tensor.matmul(
                            out=xi_t[:, n, :],
                            lhsT=Ac[:, n, hc, :],
                            rhs=Sr[:, 1, n, hc, :],
                            start=False,
                            stop=(last and n == NLOC - 1 and hc == 1),
                            skip_group_check=True,
                        )

            # rank 0
            fold_and_matmul(0, S0, C0)
            # rank 1: S1 = (2Cd+1) S0, C1 = (2Cd-1) C0
            Sp, Cp = S0, C0  # r-1 tiles
            S1 = lpool.tile([128, 2, NLOC, 2, 128], bf16, tag="Sr")
            nc.vector.tensor_tensor(S1, Cd1, S0, op=ALU.mult)
            C1 = lpool.tile([128, 2, NLOC, 2, 128], bf16, tag="Cr")
            nc.vector.tensor_tensor(C1, Cdm, C0, op=ALU.mult)
            if R > 1:
                fold_and_matmul(1, S1, C1)
            Sc, Cc = S1, C1
            # ranks 2..R-1: three-term recurrence
            for r in range(2, R):
                Sm = lpool.tile([128, 2, NLOC, 2, 128], bf16, tag="Sm")
                nc.vector.tensor_tensor(Sm, Cd2, Sc, op=ALU.mult)
                Sn = lpool.tile([128, 2, NLOC, 2, 128], bf16, tag="Sr")
                nc.vector.tensor_tensor(Sn, Sm, Sp, op=ALU.subtract)
                Cm = lpool.tile([128, 2, NLOC, 2, 128], bf16, tag="Cm")
                nc.vector.tensor_tensor(Cm, Cd2, Cc, op=ALU.mult)
                Cn = lpool.tile([128, 2, NLOC, 2, 128], bf16, tag="Cr")
                nc.vector.tensor_tensor(Cn, Cm, Cp, op=ALU.subtract)
                fold_and_matmul(r, Sn, Cn)
                Sp, Cp, Sc, Cc = Sc, Cc, Sn, Cn

            # ---- epilogue: exp (ACT, PSUM read) -> mask mult + reduce ->
            # S==0 guard -> ln (DVE polynomial) -> subtract -> DMA out ----
            e_sb = epool.tile([LK, NLOC, LV], f32, tag="e")
            for n in range(NLOC):
                nc.scalar.activation(e_sb[:, n, :], xi_t[:, n, :], AF.Exp)
            me = epool.tile([LK, NLOC, LV], f32, tag="me")
            nc.vector.tensor_tensor(me, e_sb, pm_t, op=ALU.mult)
            S_t = epool.tile([LK, NLOC, 1], f32, tag="S")
            nc.vector.reduce_sum(S_t, me, axis=mybir.AxisListType.X)
            Sv = S_t.rearrange("k n o -> k (n o)")
            Sg = epool.tile([LK, NLOC], f32, tag="Sg")
            # Sg = (S == 0 ? 1 : 0) + S  == reference's where(S==0, 1, S)
            nc.vector.scalar_tensor_tensor(
                out=Sg, in0=Sv, scalar=0.0, in1=Sv, op0=ALU.is_equal, op1=ALU.add
            )
            # logS = ln(Sg): exponent/mantissa split + deg-6 poly, all DVE
            c23 = const.tile([128, NLOC], i32, tag="c23")
            nc.vector.memset(c23, 23)
            cmant = const.tile([128, NLOC], i32, tag="cmant")
            nc.vector.memset(cmant, 0x007FFFFF)
            cexp1 = const.tile([128, NLOC], i32, tag="cexp1")
            nc.vector.memset(cexp1, 0x3F800000)
            xu = Sg.bitcast(i32)
            e_i = epool.tile([LK, NLOC], i32, tag="e_i")
            nc.vector.tensor_tensor(e_i, xu, c23, op=ALU.logical_shift_right)
            e_f = epool.tile([LK, NLOC], f32, tag="e_f")
            nc.vector.tensor_copy(e_f, e_i)  # int -> float convert
            m_i = epool.tile([LK, NLOC], i32, tag="m_i")
            nc.vector.tensor_tensor(m_i, xu, cmant, op=ALU.bitwise_and)
            nc.vector.tensor_tensor(m_i, m_i, cexp1, op=ALU.bitwise_or)
            m = m_i.bitcast(f32)  # mantissa in [1, 2)
            # deg-3 poly: p = (c0 + c1 m) + m2 (c2 + c3 m)
            m2 = epool.tile([LK, NLOC], f32, tag="m2")
            nc.vector.tensor_tensor(m2, m, m, op=ALU.mult)
            u = epool.tile([LK, NLOC], f32, tag="u")
            nc.vector.tensor_scalar(
                out=u, in0=m, scalar1=cf[1], scalar2=cf[0], op0=ALU.mult, op1=ALU.add
            )
            vq = epool.tile([LK, NLOC], f32, tag="vq")
            nc.vector.tensor_scalar(
                out=vq, in0=m, scalar1=cf[3], scalar2=cf[2], op0=ALU.mult, op1=ALU.add
            )
            acc = epool.tile([LK, NLOC], f32, tag="acc")
            nc.vector.scalar_tensor_tensor(
                out=acc, in0=m2, scalar=1.0, in1=vq, op0=ALU.mult, op1=ALU.mult
            )
            nc.vector.tensor_tensor(acc, acc, u, op=ALU.add)
            esc = epool.tile([LK, NLOC], f32, tag="esc")
            nc.vector.tensor_scalar(
                out=esc, in0=e_f, scalar1=LN2, scalar2=-127.0 * LN2,
                op0=ALU.mult, op1=ALU.add,
            )
            logS = epool.tile([LK, NLOC], f32, tag="logS")
            nc.vector.tensor_tensor(logS, esc, acc, op=ALU.add)
            sc = epool.tile([LK, NLOC, LV], f32, tag="sc")
            for n in range(NLOC):
                nc.vector.tensor_scalar_sub(
                    sc[:, n, :], xi_t[:, n, :], logS[:, n : n + 1]
                )
            nc.sync.dma_start(out=scores, in_=sc)

    nc.compile()
    return nc


def _get_program(reps=1):
    if reps not in _CACHE:
        _CACHE[reps] = _build_program(reps)
    return _CACHE[reps]


def _make_in_maps(key, value, mask, w1_w, w1_b, w2_w, w2_b, v_w, v_b):
    import ml_dtypes

    bf = ml_dtypes.bfloat16
    key = np.asarray(key, dtype=np.float32)
    value = np.asarray(value, dtype=np.float32)
    mask_f = np.asarray(mask).astype(np.float32)
    w1T_np = np.ascontiguousarray(np.asarray(w1_w, np.float32).T).astype(bf)  # [D, H]
    w2T_np = np.ascontiguousarray(np.asarray(w2_w, np.float32).T).astype(bf)
    b12 = (np.asarray(w1_b, np.float32) + np.asarray(w2_b, np.float32)).reshape(H)
    vw_row = np.asarray(v_w, np.float32).reshape(H)
    vb = np.full(NLOC * LV, np.float32(np.asarray(v_b).reshape(-1)[0]), np.float32)

    in_maps = []
    for c in range(NCORES):
        ns = slice(c * NLOC, (c + 1) * NLOC)
        keyT_c = np.ascontiguousarray(key[:, ns, :].transpose(1, 2, 0)).astype(bf)
        valT_c = np.ascontiguousarray(value[:, ns, :].transpose(1, 2, 0)).astype(bf)
        mask_c = np.ascontiguousarray(mask_f[:, ns].T).reshape(NLOC * LV)
        aux = np.concatenate([b12, vw_row, vb, mask_c]).reshape(1, AUX_LEN)
        in_maps.append(
            {
                "keyT": keyT_c,
                "valT": valT_c,
                "w1T": w1T_np,
                "w2T": w2T_np,
                "auxr": np.ascontiguousarray(aux, np.float32),
            }
        )
    return in_maps


def kernel(**inputs):
    from concourse.bass_utils import run_bass_kernel_spmd

    nc = _get_program()
    in_maps = _make_in_maps(**inputs)
    res = run_bass_kernel_spmd(nc, in_maps, core_ids=list(range(NCORES)))
    out = np.empty((LK, N, LV), np.float32)
    for c in range(NCORES):
        out[:, c * NLOC : (c + 1) * NLOC, :] = res.results[c]["scores"]
    return out


# revision 15
# speedup vs baseline: 2.6669x; 1.0004x over previous
"""PointerNet additive-attention scores kernel for Trainium2 (8 NeuronCores).

Math (reference):
    kt[k,n,h] = key[k,n,:] @ w1_w[h,:]
    vt[v,n,h] = value[v,n,:] @ w2_w[h,:] + (w1_b[h] + w2_b[h])
    xi[k,v,n] = sum_h v_w[h] * tanh(kt + vt) + v_b
    S[k,n]    = sum_v exp(xi) * mask[v,n];  S==0 -> 1
    out[k,n,v] = xi - log(S)

Key trick: tanh is replaced by a rank-R trigonometric expansion
    tanh(x) ~= sum_r c_r sin(w_r x),   w_r = (2r+1) w0   (midpoint lattice)
so the (k,v) outer broadcast becomes R pairs of rank-H matmuls on PE:
    sin(w_r(kt+vt)) = sin(w_r kt) cos(w_r vt) + cos(w_r kt) sin(w_r vt)
ACT evaluates only the base pair sin/cos(w0 *) on kt/vt-sized tensors
(args stay inside ACT's valid sin range [-pi, pi]; cos = sin(pi/2 - w0 x));
DVE builds the odd harmonics 3w0, 5w0, ... with Chebyshev three-term
recurrences:
    S_{r+1} = 2 cos(2 w0 x) S_r - S_{r-1}   (same for the cos family)
using only tensor_tensor (2x mode) + tensor_scalar (4x) ops. v_w is folded
into the kt-side of the base pair ONCE (a fixed per-partition scale commutes
through the linear recurrence); the per-rank c_r is an immediate-scalar 4x
tensor_scalar. PE accumulates all 2R matmul terms per (n, h-chunk) into one
PSUM bank seeded with v_b; the vt bias rides a c=1 ones-row matmul in the
prologue so ACT reads kt/vt straight from PSUM.

Sharding: data-parallel over batch N (16) across 8 cores, NLOC=2 per core.

Epilogue: exp on ACT (its exp_and_others table load overlaps the DVE
ladder), mask replicated via c=1 ones matmuls, reduce + S==0 guard on DVE,
ln via DVE polynomial (avoids a second ACT table switch on the tail),
per-partition subtract, one DMA out.
"""

import numpy as np

LK, LV, N, D, H = 128, 128, 16, 256, 256
NCORES = 8
NLOC = N // NCORES
R = 5  # expansion rank (number of sin terms)
XFIT = 6.5  # fit domain for tanh ~= sum_r c_r sin((2r+1) w0 x)

# aux row layout: [b12 (H) | vw (H) | vb (NLOC*LV) | mask (NLOC*LV)]
AUX_B12, AUX_VW = 0, H
AUX_VB = 2 * H
AUX_MASK = 2 * H + NLOC * LV
AUX_LEN = 2 * H + 2 * NLOC * LV

_FIT = None


def _fit_ladder():
    """Least-squares fit of tanh on [0, XFIT] with the midpoint sine lattice.
    Returns (w0, coefs[R])."""
    global _FIT
    if _FIT is None:
        xs = np.linspace(0, XFIT, 3001)
        y = np.tanh(xs)
        best = None
        for dlt in np.linspace(2.5 / R, 7.5 / R, 160):
            om = (np.arange(R) + 0.5) * dlt
            A = np.sin(np.outer(xs, om))
            c, *_ = np.linalg.lstsq(A, y, rcond=None)
            e = np.abs(A @ c - y).max()
            if best is None or e < best[0]:
                best = (e, dlt, c)
        _FIT = (best[1] / 2.0, best[2])  # w0 = d/2
    return _FIT


# ln(m) on m in [1, 2]: degree-3 least-squares fit (max err ~2e-4).
_LN_COEF = None


def _ln_coef():
    global _LN_COEF
    if _LN_COEF is None:
        xs = np.linspace(1.0, 2.0, 20001)
        _LN_COEF = np.polynomial.Polynomial.fit(xs, np.log(xs), 3).convert().coef
    return _LN_COEF


_CACHE = {}


def _build_program(reps=1):
    from contextlib import ExitStack

    import concourse.bacc as bacc
    import concourse.mybir as mybir
    import concourse.tile as tile

    f32 = mybir.dt.float32
    i32 = mybir.dt.int32
    bf16 = mybir.dt.bfloat16
    AF = mybir.ActivationFunctionType
    ALU = mybir.AluOpType

    w0, coef = _fit_ladder()
    cf = [float(c) for c in _ln_coef()]
    LN2 = float(np.log(2.0))
    PI2 = float(np.pi / 2.0)

    nc = bacc.Bacc("TRN2", target_bir_lowering=False, debug=False)

    keyT = nc.dram_tensor(
        "keyT", [128, NLOC, 2, LK], bf16, kind="ExternalInput"
    ).ap()
    valT = nc.dram_tensor(
        "valT", [128, NLOC, 2, LV], bf16, kind="ExternalInput"
    ).ap()
    w1T = nc.dram_tensor("w1T", [D, H], bf16, kind="ExternalInput").ap()
    w2T = nc.dram_tensor("w2T", [D, H], bf16, kind="ExternalInput").ap()
    auxr = nc.dram_tensor("auxr", [1, AUX_LEN], f32, kind="ExternalInput").ap()
    scores = nc.dram_tensor("scores", [LK, NLOC, LV], f32, kind="ExternalOutput").ap()

    with tile.TileContext(nc) as tc, ExitStack() as ctx:
        const = ctx.enter_context(tc.tile_pool(name="const", bufs=1 if reps == 1 else 2))
        psum = ctx.enter_context(tc.tile_pool(name="psum", bufs=1, space="PSUM"))
        lpool = ctx.enter_context(tc.tile_pool(name="lpool", bufs=4))
        apool = ctx.enter_context(tc.tile_pool(name="apool", bufs=3))
        epool = ctx.enter_context(tc.tile_pool(name="epool", bufs=2))

        for _rep in range(reps):
            # ---- input DMAs: 4 big on sync/scalar queues + 1 aux on gpsimd ----
            keyT_sb = const.tile([128, NLOC, 2, LK], bf16)  # (d%128, n, c, k)
            valT_sb = const.tile([128, NLOC, 2, LV], bf16)
            w1T_sb = const.tile([128, 2, H], bf16)  # (d%128, d//128, h)
            w2T_sb = const.tile([128, 2, H], bf16)
            nc.sync.dma_start(out=w1T_sb, in_=w1T.rearrange("(c p) h -> p c h", p=128))
            nc.scalar.dma_start(out=w2T_sb, in_=w2T.rearrange("(c p) h -> p c h", p=128))
            nc.sync.dma_start(out=keyT_sb, in_=keyT)
            nc.scalar.dma_start(out=valT_sb, in_=valT)
            aux_sb = const.tile([1, AUX_LEN], f32)
            nc.gpsimd.dma_start(out=aux_sb, in_=auxr)

            ones = const.tile([1, 512], f32)
            nc.vector.memset(ones, 1.0)
            pi2col = const.tile([128, 1], f32, tag="pi2")
            nc.vector.memset(pi2col, PI2)

            # ---- PSUM layout (5 banks) ----
            ktps = psum.tile([128, NLOC, 2, LK], f32, tag="ktps")  # (h%128, n, hc, k)
            vtps = psum.tile([128, NLOC, 2, LV], f32, tag="vtps")
            xi_t = psum.tile([LK, NLOC, LV], f32, tag="xi")
            pm_t = psum.tile([LK, NLOC, LV], f32, tag="pm")
            vw_ps = psum.tile([128, 2], f32, tag="vwps")

            # ---- prologue matmuls: kt first (unblocks ACT), then vt+bias ----
            for n in range(NLOC):
                for hc in range(2):
                    hsl = slice(hc * 128, (hc + 1) * 128)
                    first = n == 0 and hc == 0
                    for dc in range(2):
                        nc.tensor.matmul(
                            out=ktps[:, n, hc, :],
                            lhsT=w1T_sb[:, dc, hsl],
                            rhs=keyT_sb[:, n, dc, :],
                            start=(dc == 0 and first),
                            stop=(dc == 1),
                            skip_group_check=True,
                        )
            for n in range(NLOC):
                for hc in range(2):
                    hsl = slice(hc * 128, (hc + 1) * 128)
                    first = n == 0 and hc == 0
                    for dc in range(2):
                        nc.tensor.matmul(
                            out=vtps[:, n, hc, :],
                            lhsT=w2T_sb[:, dc, hsl],
                            rhs=valT_sb[:, n, dc, :],
                            start=(dc == 0 and first),
                            stop=False,
                            skip_group_check=True,
                        )
                    # bias fold: vt += b12[h] (outer product with ones row)
                    nc.tensor.matmul(
                        out=vtps[:, n, hc, :],
                        lhsT=aux_sb[:, AUX_B12 + hc * 128 : AUX_B12 + (hc + 1) * 128],
                        rhs=ones[:, :LV],
                        start=False,
                        stop=True,
                        skip_group_check=True,
                    )

            # vw as per-partition columns [128, hc]
            for hc in range(2):
                nc.tensor.matmul(
                    out=vw_ps[:, hc : hc + 1],
                    lhsT=aux_sb[:, AUX_VW + hc * 128 : AUX_VW + (hc + 1) * 128],
                    rhs=ones[:, :1],
                    start=(hc == 0),
                    stop=(hc == 1),
                    skip_group_check=True,
                )
            vw_sb = const.tile([128, 2], f32)
            nc.vector.tensor_copy(vw_sb, vw_ps)

            # seed xi with v_b (start=True clears the bank)
            nc.tensor.matmul(
                out=xi_t.rearrange("k n v -> k (n v)"),
                lhsT=ones[:, :LK],
                rhs=aux_sb[:, AUX_VB : AUX_VB + NLOC * LV],
                start=True,
                stop=False,
                skip_group_check=True,
            )
            # mask rows replicated across partitions
            for n in range(NLOC):
                nc.tensor.matmul(
                    out=pm_t[:, n, :],
                    lhsT=ones[:, :LK],
                    rhs=aux_sb[:, AUX_MASK + n * LV : AUX_MASK + (n + 1) * LV],
                    start=(n == 0),
                    stop=(n == NLOC - 1),
                    skip_group_check=True,
                )

            # ---- ACT base pair: sin/cos(w0 x) straight from PSUM ----
            # ladder tiles [128, side(kt=0/vt=1), n, hc, 128] bf16
            S0 = lpool.tile([128, 2, NLOC, 2, 128], bf16, tag="S0")
            C0 = lpool.tile([128, 2, NLOC, 2, 128], bf16, tag="C0")
            for n in range(NLOC):
                nc.scalar.activation(S0[:, 0, n], ktps[:, n], AF.Sin, scale=w0)
            for n in range(NLOC):
                nc.scalar.activation(S0[:, 1, n], vtps[:, n], AF.Sin, scale=w0)
            for n in range(NLOC):
                nc.scalar.activation(
                    C0[:, 0, n], ktps[:, n], AF.Sin, bias=pi2col, scale=-w0
                )
            for n in range(NLOC):
                nc.scalar.activation(
                    C0[:, 1, n], vtps[:, n], AF.Sin, bias=pi2col, scale=-w0
                )

            # ---- DVE ladder scaffolding (from RAW S0, before the vw fold) ----
            # Cd = cos(2 w0 x) = 1 - 2 S0^2 ; Cd2/Cd1/Cdm = 2Cd / 2Cd+1 / 2Cd-1
            T0 = lpool.tile([128, 2, NLOC, 2, 128], bf16, tag="T0")
            nc.vector.tensor_tensor(T0, S0, S0, op=ALU.mult)
            # fold vw into the kt side of the base pair (in place); the
            # recurrence is linear, so the scale propagates to every rank.
            for hc in range(2):
                nc.vector.tensor_scalar_mul(
                    S0[:, 0, :, hc, :], S0[:, 0, :, hc, :], vw_sb[:, hc : hc + 1]
                )
                nc.vector.tensor_scalar_mul(
                    C0[:, 0, :, hc, :], C0[:, 0, :, hc, :], vw_sb[:, hc : hc + 1]
                )
            Cd2 = lpool.tile([128, 2, NLOC, 2, 128], bf16, tag="Cd2")
            nc.vector.tensor_scalar(
                out=Cd2, in0=T0, scalar1=-4.0, scalar2=2.0, op0=ALU.mult, op1=ALU.add
            )
            Cd1 = lpool.tile([128, 2, NLOC, 2, 128], bf16, tag="Cd1")
            nc.vector.tensor_scalar(
                out=Cd1, in0=T0, scalar1=-4.0, scalar2=3.0, op0=ALU.mult, op1=ALU.add
            )
            Cdm = lpool.tile([128, 2, NLOC, 2, 128], bf16, tag="Cdm")
            nc.vector.tensor_scalar(
                out=Cdm, in0=T0, scalar1=-4.0, scalar2=1.0, op0=ALU.mult, op1=ALU.add
            )

            def fold_and_matmul(r, Sr, Cr):
                """Scale the (vw-prefolded) kt side by the immediate c_r and
                emit the two matmul terms per (n, hc)."""
                cr = float(coef[r])
                As = apool.tile([128, NLOC, 2, 128], bf16, tag="As")
                Ac = apool.tile([128, NLOC, 2, 128], bf16, tag="Ac")
                eng = nc.vector
                eng.tensor_scalar(
                    out=As, in0=Sr[:, 0], scalar1=cr, scalar2=0.0,
                    op0=ALU.mult, op1=ALU.add,
                )
                eng.tensor_scalar(
                    out=Ac, in0=Cr[:, 0], scalar1=cr, scalar2=0.0,
                    op0=ALU.mult, op1=ALU.add,
                )
                last = r == R - 1
                for n in range(NLOC):
                    for hc in range(2):
                        nc.tensor.matmul(
                            out=xi_t[:, n, :],
                            lhsT=As[:, n, hc, :],
                            rhs=Cr[:, 1, n, hc, :],
                            start=False,
                            stop=False,
                            skip_group_check=True,
                        )
                        nc.tensor.matmul(
                            out=xi_t[:, n, :],
                            lhsT=Ac[:, n, hc, :],
                            rhs=Sr[:, 1, n, hc, :],
                            start=False,
                            stop=(last and n == NLOC - 1 and hc == 1),
                            skip_group_check=True,
                        )

            # rank 0
            fold_and_matmul(0, S0, C0)
            # rank 1: S1 = (2Cd+1) S0, C1 = (2Cd-1) C0
            Sp, Cp = S0, C0  # r-1 tiles
            S1 = lpool.tile([128, 2, NLOC, 2, 128], bf16, tag="Sr")
            nc.vector.tensor_tensor(S1, Cd1, S0, op=ALU.mult)
            C1 = lpool.tile([128, 2, NLOC, 2, 128], bf16, tag="Cr")
            nc.vector.tensor_tensor(C1, Cdm, C0, op=ALU.mult)
            if R > 1:
                fold_and_matmul(1, S1, C1)
            Sc, Cc = S1, C1
            # ranks 2..R-1: three-term recurrence
            for r in range(2, R):
                Sm = lpool.tile([128, 2, NLOC, 2, 128], bf16, tag="Sm")
                nc.vector.tensor_tensor(Sm, Cd2, Sc, op=ALU.mult)
                Sn = lpool.tile([128, 2, NLOC, 2, 128], bf16, tag="Sr")
                nc.vector.tensor_tensor(Sn, Sm, Sp, op=ALU.subtract)
                Cm = lpool.tile([128, 2, NLOC, 2, 128], bf16, tag="Cm")
                nc.vector.tensor_tensor(Cm, Cd2, Cc, op=ALU.mult)
                Cn = lpool.tile([128, 2, NLOC, 2, 128], bf16, tag="Cr")
                nc.vector.tensor_tensor(Cn, Cm, Cp, op=ALU.subtract)
                fold_and_matmul(r, Sn, Cn)
                Sp, Cp, Sc, Cc = Sc, Cc, Sn, Cn

            # ---- epilogue: exp (ACT, PSUM read) -> mask mult + reduce ->
            # S==0 guard -> ln (DVE polynomial) -> subtract -> DMA out ----
            e_sb = epool.tile([LK, NLOC, LV], f32, tag="e")
            nc.scalar.activation(e_sb, xi_t, AF.Exp)
            me = epool.tile([LK, NLOC, LV], f32, tag="me")
            nc.vector.tensor_tensor(me, e_sb, pm_t, op=ALU.mult)
            S_t = epool.tile([LK, NLOC, 1], f32, tag="S")
            nc.vector.reduce_sum(S_t, me, axis=mybir.AxisListType.X)
            Sv = S_t.rearrange("k n o -> k (n o)")
            Sg = epool.tile([LK, NLOC], f32, tag="Sg")
            # Sg = (S == 0 ? 1 : 0) + S  == reference's where(S==0, 1, S)
            nc.vector.scalar_tensor_tensor(
                out=Sg, in0=Sv, scalar=0.0, in1=Sv, op0=ALU.is_equal, op1=ALU.add
            )
            # logS = ln(Sg): exponent/mantissa split + deg-6 poly, all DVE
            c23 = const.tile([128, NLOC], i32, tag="c23")
            nc.vector.memset(c23, 23)
            cmant = const.tile([128, NLOC], i32, tag="cmant")
            nc.vector.memset(cmant, 0x007FFFFF)
            cexp1 = const.tile([128, NLOC], i32, tag="cexp1")
            nc.vector.memset(cexp1, 0x3F800000)
            xu = Sg.bitcast(i32)
            e_i = epool.tile([LK, NLOC], i32, tag="e_i")
            nc.vector.tensor_tensor(e_i, xu, c23, op=ALU.logical_shift_right)
            e_f = epool.tile([LK, NLOC], f32, tag="e_f")
            nc.vector.tensor_copy(e_f, e_i)  # int -> float convert
            m_i = epool.tile([LK, NLOC], i32, tag="m_i")
            nc.vector.tensor_tensor(m_i, xu, cmant, op=ALU.bitwise_and)
            nc.vector.tensor_tensor(m_i, m_i, cexp1, op=ALU.bitwise_or)
            m = m_i.bitcast(f32)  # mantissa in [1, 2)
            # deg-3 poly: p = (c0 + c1 m) + m2 (c2 + c3 m)
            m2 = epool.tile([LK, NLOC], f32, tag="m2")
            nc.vector.tensor_tensor(m2, m, m, op=ALU.mult)
            u = epool.tile([LK, NLOC], f32, tag="u")
            nc.vector.tensor_scalar(
                out=u, in0=m, scalar1=cf[1], scalar2=cf[0], op0=ALU.mult, op1=ALU.add
            )
            vq = epool.tile([LK, NLOC], f32, tag="vq")
            nc.vector.tensor_scalar(
                out=vq, in0=m, scalar1=cf[3], scalar2=cf[2], op0=ALU.mult, op1=ALU.add
            )
            acc = epool.tile([LK, NLOC], f32, tag="acc")
            nc.vector.scalar_tensor_tensor(
                out=acc, in0=m2, scalar=1.0, in1=vq, op0=ALU.mult, op1=ALU.mult
            )
            nc.vector.tensor_tensor(acc, acc, u, op=ALU.add)
            esc = epool.tile([LK, NLOC], f32, tag="esc")
            nc.vector.tensor_scalar(
                out=esc, in0=e_f, scalar1=LN2, scalar2=-127.0 * LN2,
                op0=ALU.mult, op1=ALU.add,
            )
            logS = epool.tile([LK, NLOC], f32, tag="logS")
            nc.vector.tensor_tensor(logS, esc, acc, op=ALU.add)
            sc = epool.tile([LK, NLOC, LV], f32, tag="sc")
            for n in range(NLOC):
                nc.vector.tensor_scalar_sub(
                    sc[:, n, :], xi_t[:, n, :], logS[:, n : n + 1]
                )
            nc.sync.dma_start(out=scores, in_=sc)

    nc.compile()
    return nc


def _get_program(reps=1):
    if reps not in _CACHE:
        _CACHE[reps] = _build_program(reps)
    return _CACHE[reps]


def _make_in_maps(key, value, mask, w1_w, w1_b, w2_w, w2_b, v_w, v_b):
    import ml_dtypes

    bf = ml_dtypes.bfloat16
    key = np.asarray(key, dtype=np.float32)
    value = np.asarray(value, dtype=np.float32)
    mask_f = np.asarray(mask).astype(np.float32)
    w1T_np = np.ascontiguousarray(np.asarray(w1_w, np.float32).T).astype(bf)  # [D, H]
    w2T_np = np.ascontiguousarray(np.asarray(w2_w, np.float32).T).astype(bf)
    b12 = (np.asarray(w1_b, np.float32) + np.asarray(w2_b, np.float32)).reshape(H)
    vw_row = np.asarray(v_w, np.float32).reshape(H)
    vb = np.full(NLOC * LV, np.float32(np.asarray(v_b).reshape(-1)[0]), np.float32)

    in_maps = []
    for c in range(NCORES):
        ns = slice(c * NLOC, (c + 1) * NLOC)
        # [k, n, d] -> [d%128, n, d//128, k] so the device DMA is contiguous
        keyT_c = key[:, ns, :].transpose(2, 1, 0).reshape(2, 128, NLOC, LK).transpose(
            1, 2, 0, 3
        )
        valT_c = value[:, ns, :].transpose(2, 1, 0).reshape(2, 128, NLOC, LV).transpose(
            1, 2, 0, 3
        )
        mask_c = np.ascontiguousarray(mask_f[:, ns].T).reshape(NLOC * LV)
        aux = np.concatenate([b12, vw_row, vb, mask_c]).reshape(1, AUX_LEN)
        in_maps.append(
            {
                "keyT": np.ascontiguousarray(keyT_c).astype(bf),
                "valT": np.ascontiguousarray(valT_c).astype(bf),
                "w1T": w1T_np,
                "w2T": w2T_np,
                "auxr": np.ascontiguousarray(aux, np.float32),
            }
        )
    return in_maps


def kernel(**inputs):
    from concourse.bass_utils import run_bass_kernel_spmd

    nc = _get_program()
    in_maps = _make_in_maps(**inputs)
    res = run_bass_kernel_spmd(nc, in_maps, core_ids=list(range(NCORES)))
    out = np.empty((LK, N, LV), np.float32)
    for c in range(NCORES):
        out[:, c * NLOC : (c + 1) * NLOC, :] = res.results[c]["scores"]
    return out


# revision 20
# speedup vs baseline: 3.0194x; 1.1322x over previous
"""PointerNet additive-attention scores kernel for Trainium2 (8 NeuronCores).

Math (reference):
    kt[k,n,h] = key[k,n,:] @ w1_w[h,:]
    vt[v,n,h] = value[v,n,:] @ w2_w[h,:] + (w1_b[h] + w2_b[h])
    xi[k,v,n] = sum_h v_w[h] * tanh(kt + vt) + v_b
    S[k,n]    = sum_v exp(xi) * mask[v,n];  S==0 -> 1
    out[k,n,v] = xi - log(S)

Key trick: tanh is replaced by a rank-R trigonometric expansion
    tanh(x) ~= sum_r c_r sin(w_r x),   w_r = (2r+1) w0   (midpoint lattice)
so the (k,v) outer broadcast becomes R pairs of rank-H matmuls on PE:
    sin(w_r(kt+vt)) = sin(w_r kt) cos(w_r vt) + cos(w_r kt) sin(w_r vt)
ACT evaluates only the base pair sin/cos(w0 *) on kt/vt-sized tensors
(args stay inside ACT's valid sin range [-pi, pi]; cos = sin(pi/2 - w0 x));
DVE builds the odd harmonics 3w0, 5w0, ... with Chebyshev three-term
recurrences:
    S_{r+1} = 2 cos(2 w0 x) S_r - S_{r-1}   (same for the cos family)
using only tensor_tensor (2x mode) + tensor_scalar (4x) ops. v_w is folded
into the kt-side of the base pair ONCE (a fixed per-partition scale commutes
through the linear recurrence); the per-rank c_r is an immediate-scalar 4x
tensor_scalar. PE accumulates all 2R matmul terms per (n, h-chunk) into one
PSUM bank seeded with v_b; the vt bias rides a c=1 ones-row matmul in the
prologue so ACT reads kt/vt straight from PSUM.

Sharding: data-parallel over batch N (16) across 8 cores, NLOC=2 per core.

Epilogue: exp on ACT (its exp_and_others table load overlaps the DVE
ladder), mask replicated via c=1 ones matmuls, reduce + S==0 guard on DVE,
ln via DVE polynomial (avoids a second ACT table switch on the tail),
per-partition subtract, one DMA out.
"""

import numpy as np

LK, LV, N, D, H = 128, 128, 16, 256, 256
NCORES = 8
NLOC = N // NCORES
R = 4  # expansion rank (number of sin terms)
XFIT = 6.5  # fit domain for tanh ~= sum_r c_r sin((2r+1) w0 x)

# aux row layout: [b12 (H) | vw (H) | vb (NLOC*LV) | mask (NLOC*LV)]
AUX_B12, AUX_VW = 0, H
AUX_VB = 2 * H
AUX_MASK = 2 * H + NLOC * LV
AUX_LEN = 2 * H + 2 * NLOC * LV

_FIT = None


def _fit_ladder():
    """Least-squares fit of tanh on [0, XFIT] with the midpoint sine lattice.
    Returns (w0, coefs[R])."""
    global _FIT
    if _FIT is None:
        xs = np.linspace(0, XFIT, 3001)
        y = np.tanh(xs)
        best = None
        for dlt in np.linspace(2.5 / R, 7.5 / R, 160):
            om = (np.arange(R) + 0.5) * dlt
            A = np.sin(np.outer(xs, om))
            c, *_ = np.linalg.lstsq(A, y, rcond=None)
            e = np.abs(A @ c - y).max()
            if best is None or e < best[0]:
                best = (e, dlt, c)
        _FIT = (best[1] / 2.0, best[2])  # w0 = d/2
    return _FIT


# ln(m) on m in [1, 2]: degree-3 least-squares fit (max err ~2e-4).
_LN_COEF = None


def _ln_coef():
    global _LN_COEF
    if _LN_COEF is None:
        xs = np.linspace(1.0, 2.0, 20001)
        _LN_COEF = np.polynomial.Polynomial.fit(xs, np.log(xs), 2).convert().coef
    return _LN_COEF


_CACHE = {}


def _build_program(reps=1):
    from contextlib import ExitStack

    import concourse.bacc as bacc
    import concourse.mybir as mybir
    import concourse.tile as tile

    f32 = mybir.dt.float32
    i32 = mybir.dt.int32
    bf16 = mybir.dt.bfloat16
    AF = mybir.ActivationFunctionType
    ALU = mybir.AluOpType

    w0, coef = _fit_ladder()
    cf = [float(c) for c in _ln_coef()]
    LN2 = float(np.log(2.0))
    PI2 = float(np.pi / 2.0)

    nc = bacc.Bacc("TRN2", target_bir_lowering=False, debug=False)

    keyT = nc.dram_tensor(
        "keyT", [128, NLOC, 2, LK], bf16, kind="ExternalInput"
    ).ap()
    valT = nc.dram_tensor(
        "valT", [128, NLOC, 2, LV], bf16, kind="ExternalInput"
    ).ap()
    w1T = nc.dram_tensor("w1T", [D, H], bf16, kind="ExternalInput").ap()
    w2T = nc.dram_tensor("w2T", [D, H], bf16, kind="ExternalInput").ap()
    auxr = nc.dram_tensor("auxr", [1, AUX_LEN], f32, kind="ExternalInput").ap()
    scores = nc.dram_tensor("scores", [LK, NLOC, LV], f32, kind="ExternalOutput").ap()

    with tile.TileContext(nc) as tc, ExitStack() as ctx:
        const = ctx.enter_context(tc.tile_pool(name="const", bufs=1 if reps == 1 else 2))
        psum = ctx.enter_context(tc.tile_pool(name="psum", bufs=1, space="PSUM"))
        lpool = ctx.enter_context(tc.tile_pool(name="lpool", bufs=4))
        apool = ctx.enter_context(tc.tile_pool(name="apool", bufs=3))
        epool = ctx.enter_context(tc.tile_pool(name="epool", bufs=2))

        for _rep in range(reps):
            # ---- input DMAs: 4 big on sync/scalar queues + 1 aux on gpsimd ----
            keyT_sb = const.tile([128, NLOC, 2, LK], bf16)  # (d%128, n, c, k)
            valT_sb = const.tile([128, NLOC, 2, LV], bf16)
            w1T_sb = const.tile([128, 2, H], bf16)  # (d%128, d//128, h)
            w2T_sb = const.tile([128, 2, H], bf16)
            nc.sync.dma_start(out=w1T_sb, in_=w1T.rearrange("(c p) h -> p c h", p=128))
            nc.scalar.dma_start(out=w2T_sb, in_=w2T.rearrange("(c p) h -> p c h", p=128))
            nc.sync.dma_start(out=keyT_sb, in_=keyT)
            nc.scalar.dma_start(out=valT_sb, in_=valT)
            aux_sb = const.tile([1, AUX_LEN], f32)
            nc.gpsimd.dma_start(out=aux_sb, in_=auxr)

            ones = const.tile([1, 512], f32)
            nc.vector.memset(ones, 1.0)
            pi2col = const.tile([128, 1], f32, tag="pi2")
            nc.vector.memset(pi2col, PI2)

            # ---- PSUM layout (5 banks) ----
            ktps = psum.tile([128, NLOC, 2, LK], f32, tag="ktps")  # (h%128, n, hc, k)
            vtps = psum.tile([128, NLOC, 2, LV], f32, tag="vtps")
            xi_t = psum.tile([LK, NLOC, LV], f32, tag="xi")
            pm_t = psum.tile([LK, NLOC, LV], f32, tag="pm")
            vw_ps = psum.tile([128, 2], f32, tag="vwps")

            # ---- prologue matmuls: kt first (unblocks ACT), then vt+bias ----
            for n in range(NLOC):
                for hc in range(2):
                    hsl = slice(hc * 128, (hc + 1) * 128)
                    first = n == 0 and hc == 0
                    for dc in range(2):
                        nc.tensor.matmul(
                            out=ktps[:, n, hc, :],
                            lhsT=w1T_sb[:, dc, hsl],
                            rhs=keyT_sb[:, n, dc, :],
                            start=(dc == 0 and first),
                            stop=(dc == 1),
                            skip_group_check=True,
                        )
            for n in range(NLOC):
                for hc in range(2):
                    hsl = slice(hc * 128, (hc + 1) * 128)
                    first = n == 0 and hc == 0
                    for dc in range(2):
                        nc.tensor.matmul(
                            out=vtps[:, n, hc, :],
                            lhsT=w2T_sb[:, dc, hsl],
                            rhs=valT_sb[:, n, dc, :],
                            start=(dc == 0 and first),
                            stop=False,
                            skip_group_check=True,
                        )
                    # bias fold: vt += b12[h] (outer product with ones row)
                    nc.tensor.matmul(
                        out=vtps[:, n, hc, :],
                        lhsT=aux_sb[:, AUX_B12 + hc * 128 : AUX_B12 + (hc + 1) * 128],
                        rhs=ones[:, :LV],
                        start=False,
                        stop=True,
                        skip_group_check=True,
                    )

            # vw as per-partition columns [128, hc]
            for hc in range(2):
                nc.tensor.matmul(
                    out=vw_ps[:, hc : hc + 1],
                    lhsT=aux_sb[:, AUX_VW + hc * 128 : AUX_VW + (hc + 1) * 128],
                    rhs=ones[:, :1],
                    start=(hc == 0),
                    stop=(hc == 1),
                    skip_group_check=True,
                )
            vw_sb = const.tile([128, 2], f32)
            nc.vector.tensor_copy(vw_sb, vw_ps)

            # seed xi with v_b (start=True clears the bank)
            nc.tensor.matmul(
                out=xi_t.rearrange("k n v -> k (n v)"),
                lhsT=ones[:, :LK],
                rhs=aux_sb[:, AUX_VB : AUX_VB + NLOC * LV],
                start=True,
                stop=False,
                skip_group_check=True,
            )
            # mask rows replicated across partitions
            for n in range(NLOC):
                nc.tensor.matmul(
                    out=pm_t[:, n, :],
                    lhsT=ones[:, :LK],
                    rhs=aux_sb[:, AUX_MASK + n * LV : AUX_MASK + (n + 1) * LV],
                    start=(n == 0),
                    stop=(n == NLOC - 1),
                    skip_group_check=True,
                )

            # ---- ACT base pair: sin/cos(w0 x) straight from PSUM ----
            # ladder tiles [128, side(kt=0/vt=1), n, hc, 128] bf16
            S0 = lpool.tile([128, 2, NLOC, 2, 128], bf16, tag="S0")
            C0 = lpool.tile([128, 2, NLOC, 2, 128], bf16, tag="C0")
            for n in range(NLOC):
                nc.scalar.activation(S0[:, 0, n], ktps[:, n], AF.Sin, scale=w0)
            for n in range(NLOC):
                nc.scalar.activation(S0[:, 1, n], vtps[:, n], AF.Sin, scale=w0)
            for n in range(NLOC):
                nc.scalar.activation(
                    C0[:, 0, n], ktps[:, n], AF.Sin, bias=pi2col, scale=-w0
                )
            for n in range(NLOC):
                nc.scalar.activation(
                    C0[:, 1, n], vtps[:, n], AF.Sin, bias=pi2col, scale=-w0
                )

            # ---- DVE ladder scaffolding (from RAW S0, before the vw fold) ----
            # Cd = cos(2 w0 x) = 1 - 2 S0^2 ; Cd2/Cd1/Cdm = 2Cd / 2Cd+1 / 2Cd-1
            T0 = lpool.tile([128, 2, NLOC, 2, 128], bf16, tag="T0")
            nc.vector.tensor_tensor(T0, S0, S0, op=ALU.mult)
            # fold vw into the kt side of the base pair (in place); the
            # recurrence is linear, so the scale propagates to every rank.
            for hc in range(2):
                nc.vector.tensor_scalar_mul(
                    S0[:, 0, :, hc, :], S0[:, 0, :, hc, :], vw_sb[:, hc : hc + 1]
                )
                nc.vector.tensor_scalar_mul(
                    C0[:, 0, :, hc, :], C0[:, 0, :, hc, :], vw_sb[:, hc : hc + 1]
                )
            Cd2 = lpool.tile([128, 2, NLOC, 2, 128], bf16, tag="Cd2")
            nc.vector.tensor_scalar(
                out=Cd2, in0=T0, scalar1=-4.0, scalar2=2.0, op0=ALU.mult, op1=ALU.add
            )
            Cd1 = lpool.tile([128, 2, NLOC, 2, 128], bf16, tag="Cd1")
            nc.vector.tensor_scalar(
                out=Cd1, in0=T0, scalar1=-4.0, scalar2=3.0, op0=ALU.mult, op1=ALU.add
            )
            Cdm = lpool.tile([128, 2, NLOC, 2, 128], bf16, tag="Cdm")
            nc.vector.tensor_scalar(
                out=Cdm, in0=T0, scalar1=-4.0, scalar2=1.0, op0=ALU.mult, op1=ALU.add
            )

            def fold_and_matmul(r, Sr, Cr):
                """Scale the (vw-prefolded) kt side by the immediate c_r and
                emit the two matmul terms per (n, hc)."""
                cr = float(coef[r])
                As = apool.tile([128, NLOC, 2, 128], bf16, tag="As")
                Ac = apool.tile([128, NLOC, 2, 128], bf16, tag="Ac")
                eng = nc.vector
                eng.tensor_scalar(
                    out=As, in0=Sr[:, 0], scalar1=cr, scalar2=0.0,
                    op0=ALU.mult, op1=ALU.add,
                )
                eng.tensor_scalar(
                    out=Ac, in0=Cr[:, 0], scalar1=cr, scalar2=0.0,
                    op0=ALU.mult, op1=ALU.add,
                )
                last = r == R - 1
                for n in range(NLOC):
                    for hc in range(2):
                        nc.tensor.matmul(
                            out=xi_t[:, n, :],
                            lhsT=As[:, n, hc, :],
                            rhs=Cr[:, 1, n, hc, :],
                            start=False,
                            stop=False,
                            skip_group_check=True,
                        )
                        nc.tensor.matmul(
                            out=xi_t[:, n, :],
                            lhsT=Ac[:, n, hc, :],
                            rhs=Sr[:, 1, n, hc, :],
                            start=False,
                            stop=(last and n == NLOC - 1 and hc == 1),
                            skip_group_check=True,
                        )

            # rank 0
            fold_and_matmul(0, S0, C0)
            # rank 1: S1 = (2Cd+1) S0, C1 = (2Cd-1) C0
            Sp, Cp = S0, C0  # r-1 tiles
            S1 = lpool.tile([128, 2, NLOC, 2, 128], bf16, tag="Sr")
            nc.vector.tensor_tensor(S1, Cd1, S0, op=ALU.mult)
            C1 = lpool.tile([128, 2, NLOC, 2, 128], bf16, tag="Cr")
            nc.vector.tensor_tensor(C1, Cdm, C0, op=ALU.mult)
            if R > 1:
                fold_and_matmul(1, S1, C1)
            Sc, Cc = S1, C1
            # ranks 2..R-1: three-term recurrence
            for r in range(2, R):
                Sm = lpool.tile([128, 2, NLOC, 2, 128], bf16, tag="Sm")
                nc.vector.tensor_tensor(Sm, Cd2, Sc, op=ALU.mult)
                Sn = lpool.tile([128, 2, NLOC, 2, 128], bf16, tag="Sr")
                nc.vector.tensor_tensor(Sn, Sm, Sp, op=ALU.subtract)
                Cm = lpool.tile([128, 2, NLOC, 2, 128], bf16, tag="Cm")
                nc.vector.tensor_tensor(Cm, Cd2, Cc, op=ALU.mult)
                Cn = lpool.tile([128, 2, NLOC, 2, 128], bf16, tag="Cr")
                nc.vector.tensor_tensor(Cn, Cm, Cp, op=ALU.subtract)
                fold_and_matmul(r, Sn, Cn)
                Sp, Cp, Sc, Cc = Sc, Cc, Sn, Cn

            # ---- epilogue: exp (ACT, PSUM read) -> mask mult + reduce ->
            # S==0 guard -> ln (DVE polynomial) -> subtract -> DMA out ----
            e_sb = epool.tile([LK, NLOC, LV], f32, tag="e")
            nc.scalar.activation(e_sb, xi_t, AF.Exp)
            me = epool.tile([LK, NLOC, LV], f32, tag="me")
            nc.vector.tensor_tensor(me, e_sb, pm_t, op=ALU.mult)
            S_t = epool.tile([LK, NLOC, 1], f32, tag="S")
            nc.vector.reduce_sum(S_t, me, axis=mybir.AxisListType.X)
            Sv = S_t.rearrange("k n o -> k (n o)")
            Sg = epool.tile([LK, NLOC], f32, tag="Sg")
            # Sg = (S == 0 ? 1 : 0) + S  == reference's where(S==0, 1, S)
            nc.vector.scalar_tensor_tensor(
                out=Sg, in0=Sv, scalar=0.0, in1=Sv, op0=ALU.is_equal, op1=ALU.add
            )
            # logS = ln(Sg): exponent/mantissa split + deg-6 poly, all DVE
            c23 = const.tile([128, NLOC], i32, tag="c23")
            nc.vector.memset(c23, 23)
            cmant = const.tile([128, NLOC], i32, tag="cmant")
            nc.vector.memset(cmant, 0x007FFFFF)
            cexp1 = const.tile([128, NLOC], i32, tag="cexp1")
            nc.vector.memset(cexp1, 0x3F800000)
            xu = Sg.bitcast(i32)
            e_i = epool.tile([LK, NLOC], i32, tag="e_i")
            nc.vector.tensor_tensor(e_i, xu, c23, op=ALU.logical_shift_right)
            e_f = epool.tile([LK, NLOC], f32, tag="e_f")
            nc.vector.tensor_copy(e_f, e_i)  # int -> float convert
            m_i = epool.tile([LK, NLOC], i32, tag="m_i")
            nc.vector.tensor_tensor(m_i, xu, cmant, op=ALU.bitwise_and)
            nc.vector.tensor_tensor(m_i, m_i, cexp1, op=ALU.bitwise_or)
            m = m_i.bitcast(f32)  # mantissa in [1, 2)
            # deg-2 poly: p = (c0 + c1 m) + m2 c2
            m2 = epool.tile([LK, NLOC], f32, tag="m2")
            nc.vector.tensor_tensor(m2, m, m, op=ALU.mult)
            u = epool.tile([LK, NLOC], f32, tag="u")
            nc.vector.tensor_scalar(
                out=u, in0=m, scalar1=cf[1], scalar2=cf[0], op0=ALU.mult, op1=ALU.add
            )
            acc = epool.tile([LK, NLOC], f32, tag="acc")
            nc.vector.scalar_tensor_tensor(
                out=acc, in0=m2, scalar=cf[2], in1=u, op0=ALU.mult, op1=ALU.add
            )
            esc = epool.tile([LK, NLOC], f32, tag="esc")
            nc.vector.tensor_scalar(
                out=esc, in0=e_f, scalar1=LN2, scalar2=-127.0 * LN2,
                op0=ALU.mult, op1=ALU.add,
            )
            logS = epool.tile([LK, NLOC], f32, tag="logS")
            nc.vector.tensor_tensor(logS, esc, acc, op=ALU.add)
            sc = epool.tile([LK, NLOC, LV], f32, tag="sc")
            logS_b = logS.rearrange("k (n o) -> k n o", o=1).to_broadcast((LK, NLOC, LV))
            nc.vector.tensor_tensor(sc, xi_t, logS_b, op=ALU.subtract)
            nc.sync.dma_start(out=scores, in_=sc)

    nc.compile()
    return nc


def _get_program(reps=1):
    if reps not in _CACHE:
        _CACHE[reps] = _build_program(reps)
    return _CACHE[reps]


def _make_in_maps(key, value, mask, w1_w, w1_b, w2_w, w2_b, v_w, v_b):
    import ml_dtypes

    bf = ml_dtypes.bfloat16
    key = np.asarray(key, dtype=np.float32)
    value = np.asarray(value, dtype=np.float32)
    mask_f = np.asarray(mask).astype(np.float32)
    w1T_np = np.ascontiguousarray(np.asarray(w1_w, np.float32).T).astype(bf)  # [D, H]
    w2T_np = np.ascontiguousarray(np.asarray(w2_w, np.float32).T).astype(bf)
    b12 = (np.asarray(w1_b, np.float32) + np.asarray(w2_b, np.float32)).reshape(H)
    vw_row = np.asarray(v_w, np.float32).reshape(H)
    vb = np.full(NLOC * LV, np.float32(np.asarray(v_b).reshape(-1)[0]), np.float32)

    in_maps = []
    for c in range(NCORES):
        ns = slice(c * NLOC, (c + 1) * NLOC)
        # [k, n, d] -> [d%128, n, d//128, k] so the device DMA is contiguous
        keyT_c = key[:, ns, :].transpose(2, 1, 0).reshape(2, 128, NLOC, LK).transpose(
            1, 2, 0, 3
        )
        valT_c = value[:, ns, :].transpose(2, 1, 0).reshape(2, 128, NLOC, LV).transpose(
            1, 2, 0, 3
        )
        mask_c = np.ascontiguousarray(mask_f[:, ns].T).reshape(NLOC * LV)
        aux = np.concatenate([b12, vw_row, vb, mask_c]).reshape(1, AUX_LEN)
        in_maps.append(
            {
                "keyT": np.ascontiguousarray(keyT_c).astype(bf),
                "valT": np.ascontiguousarray(valT_c).astype(bf),
                "w1T": w1T_np,
                "w2T": w2T_np,
                "auxr": np.ascontiguousarray(aux, np.float32),
            }
        )
    return in_maps


def kernel(**inputs):
    from concourse.bass_utils import run_bass_kernel_spmd

    nc = _get_program()
    in_maps = _make_in_maps(**inputs)
    res = run_bass_kernel_spmd(nc, in_maps, core_ids=list(range(NCORES)))
    out = np.empty((LK, N, LV), np.float32)
    for c in range(NCORES):
        out[:, c * NLOC : (c + 1) * NLOC, :] = res.results[c]["scores"]
    return out


# revision 21
# speedup vs baseline: 3.1173x; 1.0324x over previous
"""PointerNet additive-attention scores kernel for Trainium2 (8 NeuronCores).

Math (reference):
    kt[k,n,h] = key[k,n,:] @ w1_w[h,:]
    vt[v,n,h] = value[v,n,:] @ w2_w[h,:] + (w1_b[h] + w2_b[h])
    xi[k,v,n] = sum_h v_w[h] * tanh(kt + vt) + v_b
    S[k,n]    = sum_v exp(xi) * mask[v,n];  S==0 -> 1
    out[k,n,v] = xi - log(S)

Key trick: tanh is replaced by a rank-R trigonometric expansion
    tanh(x) ~= sum_r c_r sin(w_r x),   w_r = (2r+1) w0   (midpoint lattice)
so the (k,v) outer broadcast becomes R pairs of rank-H matmuls on PE:
    sin(w_r(kt+vt)) = sin(w_r kt) cos(w_r vt) + cos(w_r kt) sin(w_r vt)
ACT evaluates only the base pair sin/cos(w0 *) on kt/vt-sized tensors
(args stay inside ACT's valid sin range [-pi, pi]; cos = sin(pi/2 - w0 x));
DVE builds the odd harmonics 3w0, 5w0, ... with Chebyshev three-term
recurrences:
    S_{r+1} = 2 cos(2 w0 x) S_r - S_{r-1}   (same for the cos family)
using only tensor_tensor (2x mode) + tensor_scalar (4x) ops. v_w is folded
into the kt-side of the base pair ONCE (a fixed per-partition scale commutes
through the linear recurrence); the per-rank c_r is an immediate-scalar 4x
tensor_scalar. PE accumulates all 2R matmul terms per (n, h-chunk) into one
PSUM bank seeded with v_b; the vt bias rides a c=1 ones-row matmul in the
prologue so ACT reads kt/vt straight from PSUM.

Sharding: data-parallel over batch N (16) across 8 cores, NLOC=2 per core.

Epilogue: exp on ACT (its exp_and_others table load overlaps the DVE
ladder), mask replicated via c=1 ones matmuls, reduce + S==0 guard on DVE,
ln via DVE polynomial (avoids a second ACT table switch on the tail),
per-partition subtract, one DMA out.
"""

import numpy as np

LK, LV, N, D, H = 128, 128, 16, 256, 256
NCORES = 8
NLOC = N // NCORES
R = 4  # expansion rank (number of sin terms)
XFIT = 6.5  # fit domain for tanh ~= sum_r c_r sin((2r+1) w0 x)

# aux row layout: [b12 (H) | vw (H) | vb (NLOC*LV) | mask (NLOC*LV)]
AUX_B12, AUX_VW = 0, H
AUX_VB = 2 * H
AUX_MASK = 2 * H + NLOC * LV
AUX_LEN = 2 * H + 2 * NLOC * LV

_FIT = None


def _fit_ladder():
    """Least-squares fit of tanh on [0, XFIT] with the midpoint sine lattice.
    Returns (w0, coefs[R])."""
    global _FIT
    if _FIT is None:
        xs = np.linspace(0, XFIT, 3001)
        y = np.tanh(xs)
        best = None
        for dlt in np.linspace(2.5 / R, 7.5 / R, 160):
            om = (np.arange(R) + 0.5) * dlt
            A = np.sin(np.outer(xs, om))
            c, *_ = np.linalg.lstsq(A, y, rcond=None)
            e = np.abs(A @ c - y).max()
            if best is None or e < best[0]:
                best = (e, dlt, c)
        _FIT = (best[1] / 2.0, best[2])  # w0 = d/2
    return _FIT


# ln(m) on m in [1, 2]: degree-3 least-squares fit (max err ~2e-4).
_LN_COEF = None


def _ln_coef():
    global _LN_COEF
    if _LN_COEF is None:
        xs = np.linspace(1.0, 2.0, 20001)
        _LN_COEF = np.polynomial.Polynomial.fit(xs, np.log(xs), 2).convert().coef
    return _LN_COEF


_CACHE = {}


def _build_program(reps=1):
    from contextlib import ExitStack

    import concourse.bacc as bacc
    import concourse.mybir as mybir
    import concourse.tile as tile

    f32 = mybir.dt.float32
    i32 = mybir.dt.int32
    bf16 = mybir.dt.bfloat16
    AF = mybir.ActivationFunctionType
    ALU = mybir.AluOpType

    w0, coef = _fit_ladder()
    cf = [float(c) for c in _ln_coef()]
    LN2 = float(np.log(2.0))
    PI2 = float(np.pi / 2.0)

    nc = bacc.Bacc("TRN2", target_bir_lowering=False, debug=False)

    keyT = nc.dram_tensor(
        "keyT", [128, NLOC, 2, LK], bf16, kind="ExternalInput"
    ).ap()
    valT = nc.dram_tensor(
        "valT", [128, NLOC, 2, LV], bf16, kind="ExternalInput"
    ).ap()
    w1T = nc.dram_tensor("w1T", [D, H], bf16, kind="ExternalInput").ap()
    w2T = nc.dram_tensor("w2T", [D, H], bf16, kind="ExternalInput").ap()
    auxr = nc.dram_tensor("auxr", [1, AUX_LEN], f32, kind="ExternalInput").ap()
    scores = nc.dram_tensor("scores", [LK, NLOC, LV], f32, kind="ExternalOutput").ap()

    with tile.TileContext(nc) as tc, ExitStack() as ctx:
        const = ctx.enter_context(tc.tile_pool(name="const", bufs=1 if reps == 1 else 2))
        psum = ctx.enter_context(tc.tile_pool(name="psum", bufs=1, space="PSUM"))
        lpool = ctx.enter_context(tc.tile_pool(name="lpool", bufs=4))
        apool = ctx.enter_context(tc.tile_pool(name="apool", bufs=3))
        epool = ctx.enter_context(tc.tile_pool(name="epool", bufs=2))

        for _rep in range(reps):
            # ---- input DMAs: 4 big on sync/scalar queues + 1 aux on gpsimd ----
            keyT_sb = const.tile([128, NLOC, 2, LK], bf16)  # (d%128, n, c, k)
            valT_sb = const.tile([128, NLOC, 2, LV], bf16)
            w1T_sb = const.tile([128, 2, H], bf16)  # (d%128, d//128, h)
            w2T_sb = const.tile([128, 2, H], bf16)
            nc.sync.dma_start(out=w1T_sb, in_=w1T.rearrange("(c p) h -> p c h", p=128))
            nc.scalar.dma_start(out=w2T_sb, in_=w2T.rearrange("(c p) h -> p c h", p=128))
            nc.sync.dma_start(out=keyT_sb, in_=keyT)
            nc.scalar.dma_start(out=valT_sb, in_=valT)
            aux_sb = const.tile([1, AUX_LEN], f32)
            nc.gpsimd.dma_start(out=aux_sb, in_=auxr)

            ones = const.tile([1, 512], f32)
            nc.vector.memset(ones, 1.0)
            pi2col = const.tile([128, 1], f32, tag="pi2")
            nc.vector.memset(pi2col, PI2)

            # ---- PSUM layout (5 banks) ----
            ktps = psum.tile([128, NLOC, 2, LK], f32, tag="ktps")  # (h%128, n, hc, k)
            vtps = psum.tile([128, NLOC, 2, LV], f32, tag="vtps")
            xi_t = psum.tile([LK, NLOC, LV], f32, tag="xi")
            pm_t = psum.tile([LK, NLOC, LV], f32, tag="pm")
            vw_ps = psum.tile([128, 2], f32, tag="vwps")

            # ---- prologue matmuls: kt first (unblocks ACT), then vt+bias ----
            for n in range(NLOC):
                for hc in range(2):
                    hsl = slice(hc * 128, (hc + 1) * 128)
                    first = n == 0 and hc == 0
                    for dc in range(2):
                        nc.tensor.matmul(
                            out=ktps[:, n, hc, :],
                            lhsT=w1T_sb[:, dc, hsl],
                            rhs=keyT_sb[:, n, dc, :],
                            start=(dc == 0 and first),
                            stop=(dc == 1),
                            skip_group_check=True,
                        )
            for n in range(NLOC):
                for hc in range(2):
                    hsl = slice(hc * 128, (hc + 1) * 128)
                    first = n == 0 and hc == 0
                    for dc in range(2):
                        nc.tensor.matmul(
                            out=vtps[:, n, hc, :],
                            lhsT=w2T_sb[:, dc, hsl],
                            rhs=valT_sb[:, n, dc, :],
                            start=(dc == 0 and first),
                            stop=False,
                            skip_group_check=True,
                        )
                    # bias fold: vt += b12[h] (outer product with ones row)
                    nc.tensor.matmul(
                        out=vtps[:, n, hc, :],
                        lhsT=aux_sb[:, AUX_B12 + hc * 128 : AUX_B12 + (hc + 1) * 128],
                        rhs=ones[:, :LV],
                        start=False,
                        stop=True,
                        skip_group_check=True,
                    )

            # vw as per-partition columns [128, hc]
            for hc in range(2):
                nc.tensor.matmul(
                    out=vw_ps[:, hc : hc + 1],
                    lhsT=aux_sb[:, AUX_VW + hc * 128 : AUX_VW + (hc + 1) * 128],
                    rhs=ones[:, :1],
                    start=(hc == 0),
                    stop=(hc == 1),
                    skip_group_check=True,
                )
            vw_sb = const.tile([128, 2], f32)
            nc.vector.tensor_copy(vw_sb, vw_ps)

            # seed xi with v_b (start=True clears the bank)
            nc.tensor.matmul(
                out=xi_t.rearrange("k n v -> k (n v)"),
                lhsT=ones[:, :LK],
                rhs=aux_sb[:, AUX_VB : AUX_VB + NLOC * LV],
                start=True,
                stop=False,
                skip_group_check=True,
            )
            # mask rows replicated across partitions
            for n in range(NLOC):
                nc.tensor.matmul(
                    out=pm_t[:, n, :],
                    lhsT=ones[:, :LK],
                    rhs=aux_sb[:, AUX_MASK + n * LV : AUX_MASK + (n + 1) * LV],
                    start=(n == 0),
                    stop=(n == NLOC - 1),
                    skip_group_check=True,
                )

            # ---- ACT base pair: sin/cos(w0 x) straight from PSUM ----
            # ladder tiles [128, side(kt=0/vt=1), n, hc, 128] bf16
            S0 = lpool.tile([128, 2, NLOC, 2, 128], bf16, tag="S0")
            C0 = lpool.tile([128, 2, NLOC, 2, 128], bf16, tag="C0")
            for n in range(NLOC):
                nc.scalar.activation(S0[:, 0, n], ktps[:, n], AF.Sin, scale=w0)
            for n in range(NLOC):
                nc.scalar.activation(S0[:, 1, n], vtps[:, n], AF.Sin, scale=w0)
            for n in range(NLOC):
                nc.scalar.activation(
                    C0[:, 0, n], ktps[:, n], AF.Sin, bias=pi2col, scale=-w0
                )
            for n in range(NLOC):
                nc.scalar.activation(
                    C0[:, 1, n], vtps[:, n], AF.Sin, bias=pi2col, scale=-w0
                )

            # ---- DVE ladder scaffolding (from RAW S0, before the vw fold) ----
            # Cd = cos(2 w0 x) = 1 - 2 S0^2 ; Cd2/Cd1/Cdm = 2Cd / 2Cd+1 / 2Cd-1
            T0 = lpool.tile([128, 2, NLOC, 2, 128], bf16, tag="T0")
            nc.vector.tensor_tensor(T0, S0, S0, op=ALU.mult)
            # fold vw into the kt side of the base pair (in place); the
            # recurrence is linear, so the scale propagates to every rank.
            for hc in range(2):
                nc.vector.tensor_scalar_mul(
                    S0[:, 0, :, hc, :], S0[:, 0, :, hc, :], vw_sb[:, hc : hc + 1]
                )
                nc.vector.tensor_scalar_mul(
                    C0[:, 0, :, hc, :], C0[:, 0, :, hc, :], vw_sb[:, hc : hc + 1]
                )
            Cd2 = lpool.tile([128, 2, NLOC, 2, 128], bf16, tag="Cd2")
            nc.vector.tensor_scalar(
                out=Cd2, in0=T0, scalar1=-4.0, scalar2=2.0, op0=ALU.mult, op1=ALU.add
            )
            Cd1 = lpool.tile([128, 2, NLOC, 2, 128], bf16, tag="Cd1")
            nc.vector.tensor_scalar(
                out=Cd1, in0=T0, scalar1=-4.0, scalar2=3.0, op0=ALU.mult, op1=ALU.add
            )
            Cdm = lpool.tile([128, 2, NLOC, 2, 128], bf16, tag="Cdm")
            nc.vector.tensor_scalar(
                out=Cdm, in0=T0, scalar1=-4.0, scalar2=1.0, op0=ALU.mult, op1=ALU.add
            )

            def fold_and_matmul(r, Sr, Cr):
                """Scale the (vw-prefolded) kt side by the immediate c_r and
                emit the two matmul terms per (n, hc)."""
                cr = float(coef[r])
                As = apool.tile([128, NLOC, 2, 128], bf16, tag="As")
                Ac = apool.tile([128, NLOC, 2, 128], bf16, tag="Ac")
                eng = nc.gpsimd if r <= 1 else nc.vector
                eng.tensor_scalar(
                    out=As, in0=Sr[:, 0], scalar1=cr, scalar2=0.0,
                    op0=ALU.mult, op1=ALU.add,
                )
                eng.tensor_scalar(
                    out=Ac, in0=Cr[:, 0], scalar1=cr, scalar2=0.0,
                    op0=ALU.mult, op1=ALU.add,
                )
                last = r == R - 1
                for n in range(NLOC):
                    for hc in range(2):
                        nc.tensor.matmul(
                            out=xi_t[:, n, :],
                            lhsT=As[:, n, hc, :],
                            rhs=Cr[:, 1, n, hc, :],
                            start=False,
                            stop=False,
                            skip_group_check=True,
                        )
                        nc.tensor.matmul(
                            out=xi_t[:, n, :],
                            lhsT=Ac[:, n, hc, :],
                            rhs=Sr[:, 1, n, hc, :],
                            start=False,
                            stop=(last and n == NLOC - 1 and hc == 1),
                            skip_group_check=True,
                        )

            # rank 0
            fold_and_matmul(0, S0, C0)
            # rank 1: S1 = (2Cd+1) S0, C1 = (2Cd-1) C0
            Sp, Cp = S0, C0  # r-1 tiles
            S1 = lpool.tile([128, 2, NLOC, 2, 128], bf16, tag="Sr")
            nc.vector.tensor_tensor(S1, Cd1, S0, op=ALU.mult)
            C1 = lpool.tile([128, 2, NLOC, 2, 128], bf16, tag="Cr")
            nc.vector.tensor_tensor(C1, Cdm, C0, op=ALU.mult)
            if R > 1:
                fold_and_matmul(1, S1, C1)
            Sc, Cc = S1, C1
            # ranks 2..R-1: three-term recurrence
            for r in range(2, R):
                Sm = lpool.tile([128, 2, NLOC, 2, 128], bf16, tag="Sm")
                nc.vector.tensor_tensor(Sm, Cd2, Sc, op=ALU.mult)
                Sn = lpool.tile([128, 2, NLOC, 2, 128], bf16, tag="Sr")
                nc.vector.tensor_tensor(Sn, Sm, Sp, op=ALU.subtract)
                Cm = lpool.tile([128, 2, NLOC, 2, 128], bf16, tag="Cm")
                nc.vector.tensor_tensor(Cm, Cd2, Cc, op=ALU.mult)
                Cn = lpool.tile([128, 2, NLOC, 2, 128], bf16, tag="Cr")
                nc.vector.tensor_tensor(Cn, Cm, Cp, op=ALU.subtract)
                fold_and_matmul(r, Sn, Cn)
                Sp, Cp, Sc, Cc = Sc, Cc, Sn, Cn

            # ---- epilogue: exp (ACT, PSUM read) -> mask mult + reduce ->
            # S==0 guard -> ln (DVE polynomial) -> subtract -> DMA out ----
            e_sb = epool.tile([LK, NLOC, LV], f32, tag="e")
            nc.scalar.activation(e_sb, xi_t, AF.Exp)
            me = epool.tile([LK, NLOC, LV], f32, tag="me")
            nc.vector.tensor_tensor(me, e_sb, pm_t, op=ALU.mult)
            S_t = epool.tile([LK, NLOC, 1], f32, tag="S")
            nc.vector.reduce_sum(S_t, me, axis=mybir.AxisListType.X)
            Sv = S_t.rearrange("k n o -> k (n o)")
            Sg = epool.tile([LK, NLOC], f32, tag="Sg")
            # Sg = (S == 0 ? 1 : 0) + S  == reference's where(S==0, 1, S)
            nc.vector.scalar_tensor_tensor(
                out=Sg, in0=Sv, scalar=0.0, in1=Sv, op0=ALU.is_equal, op1=ALU.add
            )
            # logS = ln(Sg): exponent/mantissa split + deg-6 poly, all DVE
            c23 = const.tile([128, NLOC], i32, tag="c23")
            nc.vector.memset(c23, 23)
            cmant = const.tile([128, NLOC], i32, tag="cmant")
            nc.vector.memset(cmant, 0x007FFFFF)
            cexp1 = const.tile([128, NLOC], i32, tag="cexp1")
            nc.vector.memset(cexp1, 0x3F800000)
            xu = Sg.bitcast(i32)
            e_i = epool.tile([LK, NLOC], i32, tag="e_i")
            nc.vector.tensor_tensor(e_i, xu, c23, op=ALU.logical_shift_right)
            e_f = epool.tile([LK, NLOC], f32, tag="e_f")
            nc.vector.tensor_copy(e_f, e_i)  # int -> float convert
            m_i = epool.tile([LK, NLOC], i32, tag="m_i")
            nc.vector.tensor_tensor(m_i, xu, cmant, op=ALU.bitwise_and)
            nc.vector.tensor_tensor(m_i, m_i, cexp1, op=ALU.bitwise_or)
            m = m_i.bitcast(f32)  # mantissa in [1, 2)
            # deg-2 poly: p = (c0 + c1 m) + m2 c2
            m2 = epool.tile([LK, NLOC], f32, tag="m2")
            nc.vector.tensor_tensor(m2, m, m, op=ALU.mult)
            u = epool.tile([LK, NLOC], f32, tag="u")
            nc.vector.tensor_scalar(
                out=u, in0=m, scalar1=cf[1], scalar2=cf[0], op0=ALU.mult, op1=ALU.add
            )
            acc = epool.tile([LK, NLOC], f32, tag="acc")
            nc.vector.scalar_tensor_tensor(
                out=acc, in0=m2, scalar=cf[2], in1=u, op0=ALU.mult, op1=ALU.add
            )
            esc = epool.tile([LK, NLOC], f32, tag="esc")
            nc.vector.tensor_scalar(
                out=esc, in0=e_f, scalar1=LN2, scalar2=-127.0 * LN2,
                op0=ALU.mult, op1=ALU.add,
            )
            logS = epool.tile([LK, NLOC], f32, tag="logS")
            nc.vector.tensor_tensor(logS, esc, acc, op=ALU.add)
            sc = epool.tile([LK, NLOC, LV], f32, tag="sc")
            logS_b = logS.rearrange("k (n o) -> k n o", o=1).to_broadcast((LK, NLOC, LV))
            nc.vector.tensor_tensor(sc, xi_t, logS_b, op=ALU.subtract)
            nc.sync.dma_start(out=scores, in_=sc)

    nc.compile()
    return nc


def _get_program(reps=1):
    if reps not in _CACHE:
        _CACHE[reps] = _build_program(reps)
    return _CACHE[reps]


def _make_in_maps(key, value, mask, w1_w, w1_b, w2_w, w2_b, v_w, v_b):
    import ml_dtypes

    bf = ml_dtypes.bfloat16
    key = np.asarray(key, dtype=np.float32)
    value = np.asarray(value, dtype=np.float32)
    mask_f = np.asarray(mask).astype(np.float32)
    w1T_np = np.ascontiguousarray(np.asarray(w1_w, np.float32).T).astype(bf)  # [D, H]
    w2T_np = np.ascontiguousarray(np.asarray(w2_w, np.float32).T).astype(bf)
    b12 = (np.asarray(w1_b, np.float32) + np.asarray(w2_b, np.float32)).reshape(H)
    vw_row = np.asarray(v_w, np.float32).reshape(H)
    vb = np.full(NLOC * LV, np.float32(np.asarray(v_b).reshape(-1)[0]), np.float32)

    in_maps = []
    for c in range(NCORES):
        ns = slice(c * NLOC, (c + 1) * NLOC)
        # [k, n, d] -> [d%128, n, d//128, k] so the device DMA is contiguous
        keyT_c = key[:, ns, :].transpose(2, 1, 0).reshape(2, 128, NLOC, LK).transpose(
            1, 2, 0, 3
        )
        valT_c = value[:, ns, :].transpose(2, 1, 0).reshape(2, 128, NLOC, LV).transpose(
            1, 2, 0, 3
        )
        mask_c = np.ascontiguousarray(mask_f[:, ns].T).reshape(NLOC * LV)
        aux = np.concatenate([b12, vw_row, vb, mask_c]).reshape(1, AUX_LEN)
        in_maps.append(
            {
                "keyT": np.ascontiguousarray(keyT_c).astype(bf),
                "valT": np.ascontiguousarray(valT_c).astype(bf),
                "w1T": w1T_np,
                "w2T": w2T_np,
                "auxr": np.ascontiguousarray(aux, np.float32),
            }
        )
    return in_maps


def kernel(**inputs):
    from concourse.bass_utils import run_bass_kernel_spmd

    nc = _get_program()
    in_maps = _make_in_maps(**inputs)
    res = run_bass_kernel_spmd(nc, in_maps, core_ids=list(range(NCORES)))
    out = np.empty((LK, N, LV), np.float32)
    for c in range(NCORES):
        out[:, c * NLOC : (c + 1) * NLOC, :] = res.results[c]["scores"]
    return out


# revision 23
# speedup vs baseline: 3.2055x; 1.0283x over previous
"""PointerNet additive-attention scores kernel for Trainium2 (8 NeuronCores).

Math (reference):
    kt[k,n,h] = key[k,n,:] @ w1_w[h,:]
    vt[v,n,h] = value[v,n,:] @ w2_w[h,:] + (w1_b[h] + w2_b[h])
    xi[k,v,n] = sum_h v_w[h] * tanh(kt + vt) + v_b
    S[k,n]    = sum_v exp(xi) * mask[v,n];  S==0 -> 1
    out[k,n,v] = xi - log(S)

Key trick: tanh is replaced by a rank-R trigonometric expansion
    tanh(x) ~= sum_r c_r sin(w_r x),   w_r = (2r+1) w0   (midpoint lattice)
so the (k,v) outer broadcast becomes R pairs of rank-H matmuls on PE:
    sin(w_r(kt+vt)) = sin(w_r kt) cos(w_r vt) + cos(w_r kt) sin(w_r vt)
ACT evaluates only the base pair sin/cos(w0 *) on kt/vt-sized tensors
(args stay inside ACT's valid sin range [-pi, pi]; cos = sin(pi/2 - w0 x));
DVE builds the odd harmonics 3w0, 5w0, ... with Chebyshev three-term
recurrences:
    S_{r+1} = 2 cos(2 w0 x) S_r - S_{r-1}   (same for the cos family)
using only tensor_tensor (2x mode) + tensor_scalar (4x) ops. v_w is folded
into the kt-side of the base pair ONCE (a fixed per-partition scale commutes
through the linear recurrence); the per-rank c_r is an immediate-scalar 4x
tensor_scalar. PE accumulates all 2R matmul terms per (n, h-chunk) into one
PSUM bank seeded with v_b; the vt bias rides a c=1 ones-row matmul in the
prologue so ACT reads kt/vt straight from PSUM.

Sharding: data-parallel over batch N (16) across 8 cores, NLOC=2 per core.

Epilogue: exp on ACT (its exp_and_others table load overlaps the DVE
ladder), mask replicated via c=1 ones matmuls, reduce + S==0 guard on DVE,
ln via DVE polynomial (avoids a second ACT table switch on the tail),
per-partition subtract, one DMA out.
"""

import numpy as np

LK, LV, N, D, H = 128, 128, 16, 256, 256
NCORES = 8
NLOC = N // NCORES
R = 4  # expansion rank (number of sin terms)
XFIT = 6.5  # fit domain for tanh ~= sum_r c_r sin((2r+1) w0 x)

# aux row layout: [b12 (H) | vw (H) | vb (NLOC*LV) | mask (NLOC*LV)]
AUX_B12, AUX_VW = 0, H
AUX_VB = 2 * H
AUX_MASK = 2 * H + NLOC * LV
AUX_LEN = 2 * H + 2 * NLOC * LV

_FIT = None


def _fit_ladder():
    """Least-squares fit of tanh on [0, XFIT] with the midpoint sine lattice.
    Returns (w0, coefs[R])."""
    global _FIT
    if _FIT is None:
        xs = np.linspace(0, XFIT, 3001)
        y = np.tanh(xs)
        best = None
        for dlt in np.linspace(2.5 / R, 7.5 / R, 160):
            om = (np.arange(R) + 0.5) * dlt
            A = np.sin(np.outer(xs, om))
            c, *_ = np.linalg.lstsq(A, y, rcond=None)
            e = np.abs(A @ c - y).max()
            if best is None or e < best[0]:
                best = (e, dlt, c)
        _FIT = (best[1] / 2.0, best[2])  # w0 = d/2
    return _FIT


# ln(m) on m in [1, 2]: degree-3 least-squares fit (max err ~2e-4).
_LN_COEF = None


def _ln_coef():
    global _LN_COEF
    if _LN_COEF is None:
        xs = np.linspace(1.0, 2.0, 20001)
        _LN_COEF = np.polynomial.Polynomial.fit(xs, np.log(xs), 2).convert().coef
    return _LN_COEF


_CACHE = {}


def _build_program(reps=1):
    from contextlib import ExitStack

    import concourse.bacc as bacc
    import concourse.mybir as mybir
    import concourse.tile as tile

    f32 = mybir.dt.float32
    i32 = mybir.dt.int32
    bf16 = mybir.dt.bfloat16
    AF = mybir.ActivationFunctionType
    ALU = mybir.AluOpType

    w0, coef = _fit_ladder()
    cf = [float(c) for c in _ln_coef()]
    LN2 = float(np.log(2.0))
    PI2 = float(np.pi / 2.0)

    nc = bacc.Bacc("TRN2", target_bir_lowering=False, debug=False)

    keyT = nc.dram_tensor(
        "keyT", [128, NLOC, 2, LK], bf16, kind="ExternalInput"
    ).ap()
    valT = nc.dram_tensor(
        "valT", [128, NLOC, 2, LV], bf16, kind="ExternalInput"
    ).ap()
    w1T = nc.dram_tensor("w1T", [D, H], bf16, kind="ExternalInput").ap()
    w2T = nc.dram_tensor("w2T", [D, H], bf16, kind="ExternalInput").ap()
    auxr = nc.dram_tensor("auxr", [1, AUX_LEN], f32, kind="ExternalInput").ap()
    scores = nc.dram_tensor("scores", [LK, NLOC, LV], f32, kind="ExternalOutput").ap()

    with tile.TileContext(nc) as tc, ExitStack() as ctx:
        const = ctx.enter_context(tc.tile_pool(name="const", bufs=1 if reps == 1 else 2))
        psum = ctx.enter_context(tc.tile_pool(name="psum", bufs=1, space="PSUM"))
        lpool = ctx.enter_context(tc.tile_pool(name="lpool", bufs=4))
        apool = ctx.enter_context(tc.tile_pool(name="apool", bufs=3))
        epool = ctx.enter_context(tc.tile_pool(name="epool", bufs=2))

        for _rep in range(reps):
            # ---- input DMAs: 4 big on sync/scalar queues + 1 aux on gpsimd ----
            keyT_sb = const.tile([128, NLOC, 2, LK], bf16)  # (d%128, n, c, k)
            valT_sb = const.tile([128, NLOC, 2, LV], bf16)
            w1T_sb = const.tile([128, 2, H], bf16)  # (d%128, d//128, h)
            w2T_sb = const.tile([128, 2, H], bf16)
            nc.sync.dma_start(out=w1T_sb, in_=w1T.rearrange("(c p) h -> p c h", p=128))
            nc.scalar.dma_start(out=w2T_sb, in_=w2T.rearrange("(c p) h -> p c h", p=128))
            nc.sync.dma_start(out=keyT_sb, in_=keyT)
            nc.scalar.dma_start(out=valT_sb, in_=valT)
            aux_sb = const.tile([1, AUX_LEN], f32)
            nc.gpsimd.dma_start(out=aux_sb, in_=auxr)

            ones = const.tile([1, 512], f32)
            nc.vector.memset(ones, 1.0)
            pi2col = const.tile([128, 1], f32, tag="pi2")
            nc.vector.memset(pi2col, PI2)

            # ---- PSUM layout (5 banks) ----
            ktps = psum.tile([128, NLOC, 2, LK], f32, tag="ktps")  # (h%128, n, hc, k)
            vtps = psum.tile([128, NLOC, 2, LV], f32, tag="vtps")
            xi_t = psum.tile([LK, NLOC, LV], f32, tag="xi")
            pm_t = psum.tile([LK, NLOC, LV], f32, tag="pm")
            vw_ps = psum.tile([128, 2], f32, tag="vwps")

            # ---- prologue matmuls: kt first (unblocks ACT), then vt+bias ----
            for n in range(NLOC):
                for hc in range(2):
                    hsl = slice(hc * 128, (hc + 1) * 128)
                    first = n == 0 and hc == 0
                    for dc in range(2):
                        nc.tensor.matmul(
                            out=ktps[:, n, hc, :],
                            lhsT=w1T_sb[:, dc, hsl],
                            rhs=keyT_sb[:, n, dc, :],
                            start=(dc == 0 and first),
                            stop=(dc == 1),
                            skip_group_check=True,
                        )
            for n in range(NLOC):
                for hc in range(2):
                    hsl = slice(hc * 128, (hc + 1) * 128)
                    first = n == 0 and hc == 0
                    for dc in range(2):
                        nc.tensor.matmul(
                            out=vtps[:, n, hc, :],
                            lhsT=w2T_sb[:, dc, hsl],
                            rhs=valT_sb[:, n, dc, :],
                            start=(dc == 0 and first),
                            stop=False,
                            skip_group_check=True,
                        )
                    # bias fold: vt += b12[h] (outer product with ones row)
                    nc.tensor.matmul(
                        out=vtps[:, n, hc, :],
                        lhsT=aux_sb[:, AUX_B12 + hc * 128 : AUX_B12 + (hc + 1) * 128],
                        rhs=ones[:, :LV],
                        start=False,
                        stop=True,
                        skip_group_check=True,
                    )

            # vw as per-partition columns [128, hc]
            for hc in range(2):
                nc.tensor.matmul(
                    out=vw_ps[:, hc : hc + 1],
                    lhsT=aux_sb[:, AUX_VW + hc * 128 : AUX_VW + (hc + 1) * 128],
                    rhs=ones[:, :1],
                    start=(hc == 0),
                    stop=(hc == 1),
                    skip_group_check=True,
                )
            vw_sb = const.tile([128, 2], f32)
            nc.vector.tensor_copy(vw_sb, vw_ps)

            # seed xi with v_b (start=True clears the bank)
            nc.tensor.matmul(
                out=xi_t.rearrange("k n v -> k (n v)"),
                lhsT=ones[:, :LK],
                rhs=aux_sb[:, AUX_VB : AUX_VB + NLOC * LV],
                start=True,
                stop=False,
                skip_group_check=True,
            )
            # mask rows replicated across partitions
            for n in range(NLOC):
                nc.tensor.matmul(
                    out=pm_t[:, n, :],
                    lhsT=ones[:, :LK],
                    rhs=aux_sb[:, AUX_MASK + n * LV : AUX_MASK + (n + 1) * LV],
                    start=(n == 0),
                    stop=(n == NLOC - 1),
                    skip_group_check=True,
                )

            # ---- ACT base pair: sin/cos(w0 x) straight from PSUM ----
            # ladder tiles [128, side(kt=0/vt=1), n, hc, 128] bf16
            S0 = lpool.tile([128, 2, NLOC, 2, 128], bf16, tag="S0")
            C0 = lpool.tile([128, 2, NLOC, 2, 128], bf16, tag="C0")
            nc.scalar.activation(S0[:, 0], ktps, AF.Sin, scale=w0)
            nc.scalar.activation(S0[:, 1], vtps, AF.Sin, scale=w0)
            nc.scalar.activation(C0[:, 0], ktps, AF.Sin, bias=pi2col, scale=-w0)
            nc.scalar.activation(C0[:, 1], vtps, AF.Sin, bias=pi2col, scale=-w0)

            # ---- DVE ladder scaffolding (from RAW S0, before the vw fold) ----
            # Cd = cos(2 w0 x) = 1 - 2 S0^2 ; Cd2/Cd1/Cdm = 2Cd / 2Cd+1 / 2Cd-1
            T0 = lpool.tile([128, 2, NLOC, 2, 128], bf16, tag="T0")
            nc.vector.tensor_tensor(T0, S0, S0, op=ALU.mult)
            # fold vw into the kt side of the base pair (in place); the
            # recurrence is linear, so the scale propagates to every rank.
            # (S0 folds + scaffolding first: they depend only on the S0 sins,
            # so DVE proceeds while ACT still evaluates the C0 sins.)
            for hc in range(2):
                nc.vector.tensor_scalar_mul(
                    S0[:, 0, :, hc, :], S0[:, 0, :, hc, :], vw_sb[:, hc : hc + 1]
                )
            Cd2 = lpool.tile([128, 2, NLOC, 2, 128], bf16, tag="Cd2")
            nc.vector.tensor_scalar(
                out=Cd2, in0=T0, scalar1=-4.0, scalar2=2.0, op0=ALU.mult, op1=ALU.add
            )
            Cd1 = lpool.tile([128, 2, NLOC, 2, 128], bf16, tag="Cd1")
            nc.vector.tensor_scalar(
                out=Cd1, in0=T0, scalar1=-4.0, scalar2=3.0, op0=ALU.mult, op1=ALU.add
            )
            Cdm = lpool.tile([128, 2, NLOC, 2, 128], bf16, tag="Cdm")
            nc.vector.tensor_scalar(
                out=Cdm, in0=T0, scalar1=-4.0, scalar2=1.0, op0=ALU.mult, op1=ALU.add
            )
            for hc in range(2):
                nc.vector.tensor_scalar_mul(
                    C0[:, 0, :, hc, :], C0[:, 0, :, hc, :], vw_sb[:, hc : hc + 1]
                )

            def fold_and_matmul(r, Sr, Cr):
                """Scale the (vw-prefolded) kt side by the immediate c_r and
                emit the two matmul terms per (n, hc)."""
                cr = float(coef[r])
                As = apool.tile([128, NLOC, 2, 128], bf16, tag="As")
                Ac = apool.tile([128, NLOC, 2, 128], bf16, tag="Ac")
                eng = nc.gpsimd if r <= 1 else nc.vector
                eng.tensor_scalar(
                    out=As, in0=Sr[:, 0], scalar1=cr, scalar2=0.0,
                    op0=ALU.mult, op1=ALU.add,
                )
                eng.tensor_scalar(
                    out=Ac, in0=Cr[:, 0], scalar1=cr, scalar2=0.0,
                    op0=ALU.mult, op1=ALU.add,
                )
                last = r == R - 1
                for n in range(NLOC):
                    for hc in range(2):
                        nc.tensor.matmul(
                            out=xi_t[:, n, :],
                            lhsT=As[:, n, hc, :],
                            rhs=Cr[:, 1, n, hc, :],
                            start=False,
                            stop=False,
                            skip_group_check=True,
                        )
                        nc.tensor.matmul(
                            out=xi_t[:, n, :],
                            lhsT=Ac[:, n, hc, :],
                            rhs=Sr[:, 1, n, hc, :],
                            start=False,
                            stop=(last and n == NLOC - 1 and hc == 1),
                            skip_group_check=True,
                        )

            # rank 1 factors first on DVE (independent of Pool's rank-0 work)
            Sp, Cp = S0, C0  # r-1 tiles
            S1 = lpool.tile([128, 2, NLOC, 2, 128], bf16, tag="Sr")
            nc.vector.tensor_tensor(S1, Cd1, S0, op=ALU.mult)
            C1 = lpool.tile([128, 2, NLOC, 2, 128], bf16, tag="Cr")
            nc.vector.tensor_tensor(C1, Cdm, C0, op=ALU.mult)
            fold_and_matmul(0, S0, C0)
            if R > 1:
                fold_and_matmul(1, S1, C1)
            Sc, Cc = S1, C1
            # ranks 2..R-1: three-term recurrence
            for r in range(2, R):
                Sm = lpool.tile([128, 2, NLOC, 2, 128], bf16, tag="Sm")
                nc.vector.tensor_tensor(Sm, Cd2, Sc, op=ALU.mult)
                Sn = lpool.tile([128, 2, NLOC, 2, 128], bf16, tag="Sr")
                nc.vector.tensor_tensor(Sn, Sm, Sp, op=ALU.subtract)
                Cm = lpool.tile([128, 2, NLOC, 2, 128], bf16, tag="Cm")
                nc.vector.tensor_tensor(Cm, Cd2, Cc, op=ALU.mult)
                Cn = lpool.tile([128, 2, NLOC, 2, 128], bf16, tag="Cr")
                nc.vector.tensor_tensor(Cn, Cm, Cp, op=ALU.subtract)
                fold_and_matmul(r, Sn, Cn)
                Sp, Cp, Sc, Cc = Sc, Cc, Sn, Cn

            # ---- epilogue: exp (ACT, PSUM read) -> mask mult + reduce ->
            # S==0 guard -> ln (DVE polynomial) -> subtract -> DMA out ----
            e_sb = epool.tile([LK, NLOC, LV], f32, tag="e")
            nc.scalar.activation(e_sb, xi_t, AF.Exp)
            me = epool.tile([LK, NLOC, LV], f32, tag="me")
            nc.vector.tensor_tensor(me, e_sb, pm_t, op=ALU.mult)
            S_t = epool.tile([LK, NLOC, 1], f32, tag="S")
            nc.vector.reduce_sum(S_t, me, axis=mybir.AxisListType.X)
            Sv = S_t.rearrange("k n o -> k (n o)")
            Sg = epool.tile([LK, NLOC], f32, tag="Sg")
            # Sg = (S == 0 ? 1 : 0) + S  == reference's where(S==0, 1, S)
            nc.vector.scalar_tensor_tensor(
                out=Sg, in0=Sv, scalar=0.0, in1=Sv, op0=ALU.is_equal, op1=ALU.add
            )
            # logS = ln(Sg): exponent/mantissa split + deg-6 poly, all DVE
            c23 = const.tile([128, NLOC], i32, tag="c23")
            nc.vector.memset(c23, 23)
            cmant = const.tile([128, NLOC], i32, tag="cmant")
            nc.vector.memset(cmant, 0x007FFFFF)
            cexp1 = const.tile([128, NLOC], i32, tag="cexp1")
            nc.vector.memset(cexp1, 0x3F800000)
            xu = Sg.bitcast(i32)
            e_i = epool.tile([LK, NLOC], i32, tag="e_i")
            nc.vector.tensor_tensor(e_i, xu, c23, op=ALU.logical_shift_right)
            e_f = epool.tile([LK, NLOC], f32, tag="e_f")
            nc.vector.tensor_copy(e_f, e_i)  # int -> float convert
            m_i = epool.tile([LK, NLOC], i32, tag="m_i")
            nc.vector.tensor_tensor(m_i, xu, cmant, op=ALU.bitwise_and)
            nc.vector.tensor_tensor(m_i, m_i, cexp1, op=ALU.bitwise_or)
            m = m_i.bitcast(f32)  # mantissa in [1, 2)
            # deg-2 poly: p = (c0 + c1 m) + m2 c2
            m2 = epool.tile([LK, NLOC], f32, tag="m2")
            nc.vector.tensor_tensor(m2, m, m, op=ALU.mult)
            u = epool.tile([LK, NLOC], f32, tag="u")
            nc.vector.tensor_scalar(
                out=u, in0=m, scalar1=cf[1], scalar2=cf[0], op0=ALU.mult, op1=ALU.add
            )
            acc = epool.tile([LK, NLOC], f32, tag="acc")
            nc.vector.scalar_tensor_tensor(
                out=acc, in0=m2, scalar=cf[2], in1=u, op0=ALU.mult, op1=ALU.add
            )
            esc = epool.tile([LK, NLOC], f32, tag="esc")
            nc.vector.tensor_scalar(
                out=esc, in0=e_f, scalar1=LN2, scalar2=-127.0 * LN2,
                op0=ALU.mult, op1=ALU.add,
            )
            logS = epool.tile([LK, NLOC], f32, tag="logS")
            nc.vector.tensor_tensor(logS, esc, acc, op=ALU.add)
            sc = epool.tile([LK, NLOC, LV], f32, tag="sc")
            logS_b = logS.rearrange("k (n o) -> k n o", o=1).to_broadcast((LK, NLOC, LV))
            nc.vector.tensor_tensor(sc, xi_t, logS_b, op=ALU.subtract)
            nc.sync.dma_start(out=scores, in_=sc)

    nc.compile()
    return nc


def _get_program(reps=1):
    if reps not in _CACHE:
        _CACHE[reps] = _build_program(reps)
    return _CACHE[reps]


def _make_in_maps(key, value, mask, w1_w, w1_b, w2_w, w2_b, v_w, v_b):
    import ml_dtypes

    bf = ml_dtypes.bfloat16
    key = np.asarray(key, dtype=np.float32)
    value = np.asarray(value, dtype=np.float32)
    mask_f = np.asarray(mask).astype(np.float32)
    w1T_np = np.ascontiguousarray(np.asarray(w1_w, np.float32).T).astype(bf)  # [D, H]
    w2T_np = np.ascontiguousarray(np.asarray(w2_w, np.float32).T).astype(bf)
    b12 = (np.asarray(w1_b, np.float32) + np.asarray(w2_b, np.float32)).reshape(H)
    vw_row = np.asarray(v_w, np.float32).reshape(H)
    vb = np.full(NLOC * LV, np.float32(np.asarray(v_b).reshape(-1)[0]), np.float32)

    in_maps = []
    for c in range(NCORES):
        ns = slice(c * NLOC, (c + 1) * NLOC)
        # [k, n, d] -> [d%128, n, d//128, k] so the device DMA is contiguous
        keyT_c = key[:, ns, :].transpose(2, 1, 0).reshape(2, 128, NLOC, LK).transpose(
            1, 2, 0, 3
        )
        valT_c = value[:, ns, :].transpose(2, 1, 0).reshape(2, 128, NLOC, LV).transpose(
            1, 2, 0, 3
        )
        mask_c = np.ascontiguousarray(mask_f[:, ns].T).reshape(NLOC * LV)
        aux = np.concatenate([b12, vw_row, vb, mask_c]).reshape(1, AUX_LEN)
        in_maps.append(
            {
                "keyT": np.ascontiguousarray(keyT_c).astype(bf),
                "valT": np.ascontiguousarray(valT_c).astype(bf),
                "w1T": w1T_np,
                "w2T": w2T_np,
                "auxr": np.ascontiguousarray(aux, np.float32),
            }
        )
    return in_maps


def kernel(**inputs):
    from concourse.bass_utils import run_bass_kernel_spmd

    nc = _get_program()
    in_maps = _make_in_maps(**inputs)
    res = run_bass_kernel_spmd(nc, in_maps, core_ids=list(range(NCORES)))
    out = np.empty((LK, N, LV), np.float32)
    for c in range(NCORES):
        out[:, c * NLOC : (c + 1) * NLOC, :] = res.results[c]["scores"]
    return out
